# revision 1
# baseline (speedup 1.0000x reference)
"""DeepInfoMax loss kernel for 8 Trainium2 NeuronCores.

Strategy (hardcoded for B=8192, d=1024, n=16):
  - Data-parallel over batch: core c gets rows [c*1024, (c+1)*1024), plus ONE
    overlap row ((c+1)*1024 % B) of M so the global roll (M_prime) is exact.
  - Activations are kept feature-major ([features, batch]) on-chip so weights
    are the stationary matmul operand.
  - Algebraic sharing: net(M) (global discriminator's M-branch) and the
    y-contribution of the local experts' first layer commute with the batch
    roll, so both are computed ONCE and re-sliced for the joint/marginal pass.
  - bf16 matmuls with fp32 PSUM accumulation; softplus = ln(1+exp(x)) on the
    scalar engine with fused accumulation into per-core partial sums.
  - Host combines 8x [128,8] partial-sum tiles into the final scalar.
"""

import numpy as np
import ml_dtypes

B = 8192
D = 1024
NI = 16
DN = D // NI  # 64
NC = 8
BS = B // NC  # 1024
BSP = BS + 1  # 1025 (overlap col for the exact roll)
ALPHA = 0.5
BETA = 1.0

# column chunks over the 1025-wide (producer) and 1024-wide (consumer) phases
CH_P = [(0, 342), (342, 342), (684, 341)]
CH_C = [(0, 512), (512, 512)]

BF = ml_dtypes.bfloat16
F8 = ml_dtypes.float8_e4m3
WSC = 64.0

_RUNNER = None  # cached (nc, run) so repeated kernel() calls don't rebuild


def _build_nc():
    import concourse.bass as bass
    import concourse.tile as tile
    import concourse.mybir as mybir
    from concourse import bacc
    from contextlib import ExitStack

    bf = mybir.dt.bfloat16
    f32 = mybir.dt.float32
    AF = mybir.ActivationFunctionType
    OP = mybir.AluOpType

    nc = bacc.Bacc()

    # ---- DRAM I/O ----
    f8 = mybir.dt.float8e4
    ytd = nc.dram_tensor("ytd", [4, 128, 2 * 1040], f8, kind="ExternalInput")
    mtd = nc.dram_tensor("mtd", [4, 128, 2 * 1040], f8, kind="ExternalInput")
    m3d = nc.dram_tensor("m3d", [16, 128, 2 * 1040], f8, kind="ExternalInput")
    gw0d = nc.dram_tensor("gw0d", [4, 128, 2 * D], f8, kind="ExternalInput")
    gw1d = nc.dram_tensor("gw1d", [4, 128, 2 * D], f8, kind="ExternalInput")
    bxd = nc.dram_tensor("bxd", [4, 128, 2 * 2176], f8, kind="ExternalInput")
    acatd = nc.dram_tensor("acatd", [128, 16 * 256], f8, kind="ExternalInput")
    w2sp = nc.dram_tensor("w2sp", [128, 2048], bf, kind="ExternalInput")
    w3sp = nc.dram_tensor("w3sp", [128, NI], bf, kind="ExternalInput")
    l0whp = nc.dram_tensor("l0whp", [128, 1024], bf, kind="ExternalInput")
    l1wp = nc.dram_tensor("l1wp", [128, 128], bf, kind="ExternalInput")
    l2wp = nc.dram_tensor("l2wp", [128, 1], bf, kind="ExternalInput")
    gb0c = nc.dram_tensor("gb0c", [128, 8], f32, kind="ExternalInput")
    gb1c = nc.dram_tensor("gb1c", [128, 8], f32, kind="ExternalInput")
    lb1c = nc.dram_tensor("lb1c", [128, NI], f32, kind="ExternalInput")
    lb2c = nc.dram_tensor("lb2c", [128, NI], f32, kind="ExternalInput")
    onesr = nc.dram_tensor("onesr", [1, 128], bf, kind="ExternalInput")
    b3r8 = nc.dram_tensor("b3r8", [1, 128], bf, kind="ExternalInput")
    l0bc = nc.dram_tensor("l0bc", [128, 1], f32, kind="ExternalInput")
    l1bc = nc.dram_tensor("l1bc", [128, 1], f32, kind="ExternalInput")
    l2bc2 = nc.dram_tensor("l2bc2", [128, 2], f32, kind="ExternalInput")
    acc = nc.dram_tensor("acc", [128, 8], f32, kind="ExternalOutput")

    with tile.TileContext(nc) as tc, ExitStack() as ctx:
        pconst = ctx.enter_context(tc.tile_pool(name="const", bufs=1))
        pgw = ctx.enter_context(tc.tile_pool(name="gw", bufs=8))
        pbx = ctx.enter_context(tc.tile_pool(name="bx", bufs=4))
        pi8 = ctx.enter_context(tc.tile_pool(name="i8", bufs=8))
        pac = ctx.enter_context(tc.tile_pool(name="ac", bufs=1))
        p25 = ctx.enter_context(tc.tile_pool(name="t25", bufs=8))
        pze = ctx.enter_context(tc.tile_pool(name="ze", bufs=16))
        pgy = ctx.enter_context(tc.tile_pool(name="gy", bufs=1))
        ptr4 = ctx.enter_context(tc.tile_pool(name="tr4", bufs=4))
        ptr2 = ctx.enter_context(tc.tile_pool(name="tr2", bufs=4))
        ptr1 = ctx.enter_context(tc.tile_pool(name="tr1", bufs=1))
        ppm = ctx.enter_context(tc.tile_pool(name="pm", bufs=5, space="PSUM"))
        ppp = ctx.enter_context(tc.tile_pool(name="pp", bufs=1, space="PSUM"))

        # ---- phase A inputs (fp8 DoubleRow layout, chunk-granular DMAs
        # ordered by first need: chunk 0 of every k2 first) ----
        gw0_sb = []
        mt_sb = []
        for k2 in range(4):
            t = pi8.tile([128, 2 * 1040], f8, tag="i8", name=f"mtd_{k2}")
            mt_sb.append(t)
            g = pgw.tile([128, 2 * D], f8, tag="gw", name=f"gw0_{k2}")
            gw0_sb.append(g)
        c0, cw = CH_P[0]
        for k2 in range(4):
            nc.sync.dma_start(gw0_sb[k2][:], gw0d[k2, :, :])
            nc.sync.dma_start(
                mt_sb[k2].rearrange("p (ko b) -> p ko b", ko=2)[
                    :, :, c0:c0 + cw],
                mtd[k2, :, :].rearrange("p (ko b) -> p ko b", ko=2)[
                    :, :, c0:c0 + cw])
        for ci, (c0, cw) in enumerate(CH_P[1:]):
            for k2 in range(4):
                nc.sync.dma_start(
                    mt_sb[k2].rearrange("p (ko b) -> p ko b", ko=2)[
                        :, :, c0:c0 + cw],
                    mtd[k2, :, :].rearrange("p (ko b) -> p ko b", ko=2)[
                        :, :, c0:c0 + cw])

        # ---- constants into SBUF ----
        def cload(dram, shape, dt):
            t = pconst.tile(shape, dt, tag=dram.name, name=dram.name + "_sb")
            nc.gpsimd.dma_start(t[:], dram[:])
            return t

        w3s_sb = cload(w3sp, [128, NI], bf)
        l1w_sb = cload(l1wp, [128, 128], bf)
        l2w_sb = cload(l2wp, [128, 1], bf)
        l0wh_sb = cload(l0whp, [128, 1024], bf)
        w2s_sb = pac.tile([128, 2048], bf, tag="w2s")
        nc.gpsimd.dma_start(w2s_sb[:], w2sp[:])
        gb0_sb = cload(gb0c, [128, 8], f32)
        gb1_sb = cload(gb1c, [128, 8], f32)
        lb1_sb = cload(lb1c, [128, NI], f32)
        lb2_sb = cload(lb2c, [128, NI], f32)
        ones_sb = cload(onesr, [1, 128], bf)
        b3r8_sb = cload(b3r8, [1, 128], bf)
        l0b_sb = cload(l0bc, [128, 1], f32)
        l1b_sb = cload(l1bc, [128, 1], f32)
        l2b_sb = cload(l2bc2, [128, 2], f32)
        acc_sb = pconst.tile([128, 8], f32, tag="acc")
        nc.vector.memset(acc_sb[:], 0.0)

        # ze tiles: plane0 (cols 0..1039) = y_part (written in phase C);
        # planes 1/2 = expert M3 joint/marg, DMA'd upfront on gpsimd queue
        ze_sb = []
        for m in range(16):
            t = pze.tile([128, 4160], f8, tag="ze", name=f"ze_{m}")
            nc.gpsimd.dma_start(t[:, 1040:3120], m3d[m, :, :])
            ze_sb.append(t)

        # ---- phase A: h_g = relu(M @ gw0 + gb0), fp8 DoubleRow, 1025 cols ----
        # h_g stored fp8 in DoubleRow layout: tile k2 holds m-tiles (2k2, 2k2+1)
        DR = mybir.MatmulPerfMode.DoubleRow
        hg_sb = []
        for k2 in range(4):
            t = pi8.tile([128, 2 * 1040], f8, tag="i8", name=f"hgd_{k2}")
            hg_sb.append(t)
        for m in range(8):
            for (c0, cw) in CH_P:
                ps = ppm.tile([128, 512], f32, tag="pm")
                for k2 in range(4):
                    nc.tensor.matmul(
                        ps[:, :cw],
                        gw0_sb[k2].rearrange("p (ko m) -> p ko m", ko=2)[
                            :, :, m * 128:(m + 1) * 128],
                        mt_sb[k2].rearrange("p (ko b) -> p ko b", ko=2)[
                            :, :, c0:c0 + cw],
                        start=(k2 == 0), stop=(k2 == 3), perf_mode=DR,
                    )
                nc.scalar.activation(
                    hg_sb[m // 2][:, (m % 2) * 1040 + c0:(m % 2) * 1040 + c0 + cw],
                    ps[:, :cw], AF.Relu,
                    bias=gb0_sb[:, m:m + 1], scale=1.0 / WSC,
                )

        # prefetch gw1 (k2-granular)
        gw1_sb = []
        for k2 in range(4):
            g = pgw.tile([128, 2 * D], f8, tag="gw", name=f"gw1_{k2}")
            nc.sync.dma_start(g[:], gw1d[k2, :, :])
            gw1_sb.append(g)

        # ---- phase B: hM = h_g @ gw1 + gb1 (no relu), 1025 cols ----
        hm_sb = []
        for m in range(8):
            t = p25.tile([128, BSP], bf, tag="t25", name=f"hm_{m}")
            hm_sb.append(t)
        for m in range(8):
            for (c0, cw) in CH_P:
                ps = ppm.tile([128, 512], f32, tag="pm")
                for k2 in range(4):
                    nc.tensor.matmul(
                        ps[:, :cw],
                        gw1_sb[k2].rearrange("p (ko m) -> p ko m", ko=2)[
                            :, :, m * 128:(m + 1) * 128],
                        hg_sb[k2].rearrange("p (ko b) -> p ko b", ko=2)[
                            :, :, c0:c0 + cw],
                        start=(k2 == 0), stop=(k2 == 3), perf_mode=DR,
                    )
                nc.scalar.activation(
                    hm_sb[m][:, c0:c0 + cw], ps[:, :cw], AF.Identity,
                    bias=gb1_sb[:, m:m + 1], scale=1.0 / WSC,
                )

        # prefetch phase C inputs (fp8 DoubleRow, k2-granular)
        bx_sb = []
        for k2 in range(4):
            t = pbx.tile([128, 2 * 2176], f8, tag="bx", name=f"bxd_{k2}")
            nc.sync.dma_start(t[:], bxd[k2, :, :])
            bx_sb.append(t)
        yt_sb = []
        for k2 in range(4):
            t = pi8.tile([128, 2 * 1040], f8, tag="i8", name=f"ytd_{k2}")
            nc.sync.dma_start(t[:], ytd[k2, :, :])
            yt_sb.append(t)
        acat_sb = pac.tile([128, 16 * 256], f8, tag="acat")
        nc.gpsimd.dma_start(acat_sb[:], acatd[:])

        # ---- phase C: y_part (m 0..15) and gy (m 16), 1024 cols ----
        # yp[m] = (y @ Bcat)[:, m-block]^T ; gy = y @ l0w[:1024] + l0b (fp32)
        gy_sb = pgy.tile([128, BS], f32, tag="gy")
        yp_sb = [None] * 16

        def emit_C_m(m):
            if m < 16:
                yp_sb[m] = ze_sb[m]
            for (c0, cw) in CH_C:
                ps = ppm.tile([128, 512], f32, tag="pm")
                for k2 in range(4):
                    nc.tensor.matmul(
                        ps[:, :cw],
                        bx_sb[k2].rearrange("p (ko m) -> p ko m", ko=2)[
                            :, :, m * 128:(m + 1) * 128],
                        yt_sb[k2].rearrange("p (ko b) -> p ko b", ko=2)[
                            :, :, c0:c0 + cw],
                        start=(k2 == 0), stop=(k2 == 3), perf_mode=DR,
                    )
                if m < 16:
                    # fp8 plane 0 of ze[m]: y_part at true scale
                    nc.vector.tensor_scalar_mul(
                        yp_sb[m][:, c0:c0 + cw], ps[:, :cw], 1.0 / WSC)
                else:
                    nc.scalar.activation(
                        gy_sb[:, c0:c0 + cw], ps[:, :cw], AF.Identity,
                        bias=l0b_sb[:, 0:1], scale=1.0 / WSC,
                    )

        emit_C_m(16)  # gy first (F depends on it)
        for m in range(4):
            emit_C_m(m)

        # local scores: psum_p[p][:, e*8+bt] = s(batch bt*128+row, expert e)
        psum_p = [ppp.tile([128, 128], f32, tag=f"pp{p}", name=f"psum_p{p}")
                  for p in range(2)]
        psum_g = ppp.tile([128, 16], f32, tag="pg", name="psum_g")

        # ---- phase F: global discriminator, both passes ----
        for p in range(2):
            off = p
            sgn = -1.0 if p == 0 else 1.0
            for ci, (c0, cw) in enumerate(CH_C):
                ps = ppm.tile([128, 512], f32, tag="pm")
                for k in range(8):
                    nc.tensor.matmul(
                        ps[:, :cw],
                        l0wh_sb[:, k * 128:(k + 1) * 128],
                        hm_sb[k][:, off + c0:off + c0 + cw],
                        start=(k == 0), stop=(k == 7),
                    )
                z0 = ptr2.tile([128, 512], bf, tag="z0")
                nc.vector.scalar_tensor_tensor(
                    z0[:, :cw], ps[:, :cw], 0.0, gy_sb[:, c0:c0 + cw],
                    op0=OP.add, op1=OP.add)
                h0 = ptr2.tile([128, 512], bf, tag="h0")
                nc.scalar.activation(h0[:, :cw], z0[:, :cw], AF.Relu)
                ps1 = ppm.tile([128, 512], f32, tag="pm")
                nc.tensor.matmul(
                    ps1[:, :cw], l1w_sb[:], h0[:, :cw], start=True, stop=True)
                h1g = ptr2.tile([128, 512], bf, tag="h1g")
                nc.scalar.activation(
                    h1g[:, :cw], ps1[:, :cw], AF.Relu, bias=l1b_sb[:, 0:1])
                for bti in range(4):
                    bt = ci * 4 + bti
                    nc.tensor.matmul(
                        psum_g[:, p * 8 + bt:p * 8 + bt + 1],
                        h1g[:, bti * 128:(bti + 1) * 128],
                        l2w_sb[:, 0:1],
                        start=True, stop=True,
                    )

        # ---- expert phase: z1 = yp + mA (+b1) built in PSUM ----
        # psum := I.T @ yp_chunk  (seed)  +  A_e.T @ M3_chunk ; ACT evicts
        # relu(psum + b1) -> h1; L2 on PE; DVE evicts relu(z2+b2) -> h2;
        # transposed L3 into psum_p columns.
        for e in range(NI):
            # emit C for expert e+4 lazily: overlaps y_part matmuls with experts
            if e + 4 < 16 and yp_sb[e + 4] is None:
                emit_C_m(e + 4)

            for p in range(2):
                # one DoubleRow matmul: plane0 = WSC*I x y_part,
                # plane1 = WSC*A_e x M3 (joint: +1040 stride, marg: +2080)
                span2 = 2080 * (1 + p)
                h1 = ptr4.tile([128, BS], bf, tag="h1", name=f"h1_{e}_{p}")
                for ci, (c0, cw) in enumerate(CH_C):
                    ps = ppm.tile([128, 512], f32, tag="pm")
                    nc.tensor.matmul(
                        ps[:, :cw],
                        acat_sb[:, e * 256:(e + 1) * 256].rearrange(
                            "p (ko m) -> p ko m", ko=2),
                        yp_sb[e][:, 0:span2].rearrange(
                            "p (ko b) -> p ko b", ko=2)[:, :, c0:c0 + cw],
                        start=True, stop=True, perf_mode=DR,
                    )
                    nc.scalar.activation(
                        h1[:, c0:c0 + cw], ps[:, :cw], AF.Relu,
                        bias=lb1_sb[:, e:e + 1], scale=1.0 / WSC)
                h2 = ptr2.tile([128, BS], bf, tag="h2", name=f"h2_{e}_{p}")
                for ci, (c0, cw) in enumerate(CH_C):
                    ps2 = ppm.tile([128, 512], f32, tag="pm")
                    nc.tensor.matmul(
                        ps2[:, :cw],
                        w2s_sb[:, e * 128:(e + 1) * 128],
                        h1[:, c0:c0 + cw],
                        start=True, stop=True,
                    )
                    nc.vector.tensor_scalar(
                        h2[:, c0:c0 + cw], ps2[:, :cw],
                        lb2_sb[:, e:e + 1], 0.0, op0=OP.add, op1=OP.max)
                # L3 transposed: h2 b-tile stationary, w3 col moving;
                # score for (expert e, batch tile bt) -> psum_p col e*8+bt.
                # b3[e] seeded by a K=1 rank-1 matmul (ones x b3r8).
                nc.tensor.matmul(
                    psum_p[p][:, e * 8:(e + 1) * 8],
                    ones_sb[0:1, :],
                    b3r8_sb[0:1, e * 8:(e + 1) * 8],
                    start=True, stop=False, skip_group_check=True,
                )
                for bt in range(8):
                    nc.tensor.matmul(
                        psum_p[p][:, e * 8 + bt:e * 8 + bt + 1],
                        h2[:, bt * 128:(bt + 1) * 128],
                        w3s_sb[:, e:e + 1],
                        start=False, stop=True, skip_group_check=True,
                    )

        # ---- local softplus reduction: acc col p ----
        for p in range(2):
            sgn = -1.0 if p == 0 else 1.0
            exl = ptr1.tile([128, 128], f32, tag="exl", name=f"exl{p}")
            nc.scalar.activation(exl[:], psum_p[p][:], AF.Exp, scale=sgn)
            spl = ptr1.tile([128, 128], f32, tag="spl", name=f"spl{p}")
            nc.scalar.activation(
                spl[:], exl[:], AF.Ln, bias=1.0,
                accum_out=acc_sb[:, p:p + 1])

        # ---- global softplus reduction: acc col 2+p ----
        for p in range(2):
            sgn = -1.0 if p == 0 else 1.0
            exg = ptr1.tile([128, 16], f32, tag="exg", name=f"exg{p}")
            nc.scalar.activation(
                exg[:, :8], psum_g[:, p * 8:(p + 1) * 8], AF.Exp,
                scale=sgn, bias=l2b_sb[:, p:p + 1])
            spg = ptr1.tile([128, 16], f32, tag="spg", name=f"spg{p}")
            nc.scalar.activation(
                spg[:, :8], exg[:, :8], AF.Ln, bias=1.0,
                accum_out=acc_sb[:, 2 + p:3 + p])

        # ---- output ----
        nc.sync.dma_start(acc[:], acc_sb[:])

    nc.finalize()
    return nc


def _acatd(lW1):
    # [128, 16*256] fp8: per expert e: cols e*256..+127 = WSC*I128,
    # cols +128..+255 = WSC*A_e (rows 0..63), rows 64..127 zero.
    out = np.zeros((128, NI * 256), np.float32)
    eye = np.eye(128, dtype=np.float32) * WSC
    lW1 = np.asarray(lW1, np.float32)
    for e in range(NI):
        out[:, e * 256:e * 256 + 128] = eye
        out[:DN, e * 256 + 128:(e + 1) * 256] = lW1[e, :DN, :] * WSC
    return np.clip(out, -240, 240).astype(F8)


def _prep_shared(inputs):
    """Weight repack (identical for all cores), fp32 -> bf16."""
    f32 = np.float32
    gw0 = np.asarray(inputs["gw0"], f32)
    gw1 = np.asarray(inputs["gw1"], f32)
    l0w = np.asarray(inputs["l0w"], f32)
    l1w = np.asarray(inputs["l1w"], f32)
    l2w = np.asarray(inputs["l2w"], f32)
    lW1 = np.asarray(inputs["lW1"], f32)
    lW2 = np.asarray(inputs["lW2"], f32)
    lW3 = np.asarray(inputs["lW3"], f32)
    gb0 = np.asarray(inputs["gb0"], f32)
    gb1 = np.asarray(inputs["gb1"], f32)
    l0b = np.asarray(inputs["l0b"], f32)
    l1b = np.asarray(inputs["l1b"], f32)
    l2b = np.asarray(inputs["l2b"], f32)
    lb1 = np.asarray(inputs["lb1"], f32)
    lb2 = np.asarray(inputs["lb2"], f32)
    lb3 = np.asarray(inputs["lb3"], f32)

    def pk(a, kb):  # [K, N] -> [128, (K/128)*N] col-block k = rows k*128..
        K, N = a.shape
        return np.ascontiguousarray(
            a.reshape(K // 128, 128, N).transpose(1, 0, 2).reshape(128, -1))

    def _unused_dbl_marker(a):
        pass

    def dbl(a, scale=1.0, pad=None):
        # [1024, N] -> [4, 128, 2*Np] fp8 DoubleRow: f = k2*256 + ko*128 + ki
        K, N = a.shape
        Np = N if pad is None else pad
        out = np.zeros((4, 2, 128, Np), np.float32)
        out[:, :, :, :N] = a.reshape(4, 2, 128, N) * scale
        out = out.transpose(0, 2, 1, 3).reshape(4, 128, 2 * Np)
        return np.clip(out, -240.0, 240.0).astype(F8)

    bcatx = np.concatenate(
        [lW1[:, DN:, :].transpose(1, 0, 2).reshape(D, NI * 128), l0w[:D]], axis=1)
    sh = {
        "gw0d": dbl(gw0, WSC),
        "gw1d": dbl(gw1, WSC),
        "bxd": dbl(bcatx, WSC, pad=2176),
        "acatd": _acatd(lW1),
        "w2sp": np.ascontiguousarray(
            lW2.transpose(1, 0, 2).reshape(128, NI * 128)).astype(BF),
        "w3sp": np.ascontiguousarray(lW3[:, :, 0].T).astype(BF),
        "l0whp": pk(l0w[D:], 128).astype(BF),
        "l1wp": l1w.astype(BF),
        "l2wp": l2w.astype(BF),
        "gb0c": np.ascontiguousarray(gb0.reshape(8, 128).T),
        "gb1c": np.ascontiguousarray(gb1.reshape(8, 128).T),
        "lb1c": np.ascontiguousarray(lb1.T),
        "lb2c": np.ascontiguousarray(lb2.T),
        "onesr": np.ones((1, 128), BF),
        "b3r8": np.repeat(lb3[:, 0], 8)[None, :].astype(BF),
        "l0bc": np.ascontiguousarray(l0b[:, None]),
        "l1bc": np.ascontiguousarray(l1b[:, None]),
        "l2bc2": np.ascontiguousarray(
            np.stack([np.full(128, -l2b[0], f32),
                      np.full(128, l2b[0], f32)], axis=1)),
    }
    return sh


def _prep_core(inputs, c):
    f32 = np.float32
    y = np.asarray(inputs["y"], f32)
    M = np.asarray(inputs["M"], f32)
    r0 = c * BS
    rows = np.arange(r0, r0 + BSP) % B  # 1025 rows incl. overlap
    Ms = M[rows]  # [1025, 1024]
    ys = y[r0:r0 + BS]  # [1024, 1024]
    # expert-major M: m3t[e, p, b] = Ms[b, p*16+e]
    m3t = np.ascontiguousarray(
        Ms.reshape(BSP, DN, NI).transpose(2, 1, 0))  # [16,64,1025] f32

    # m3d[e]: [128, 2*1040]: cols 0..1039 joint plane (b 0..1023),
    # cols 1040.. marg plane (b 1..1024); rows 64..127 zero.
    m3dd = np.zeros((NI, 128, 2 * 1040), np.float32)
    m3dd[:, :DN, 0:BS] = m3t[:, :, 0:BS]
    m3dd[:, :DN, 1040:1040 + BS] = m3t[:, :, 1:BS + 1]
    m3dd = np.clip(m3dd, -240, 240).astype(F8)

    def dbl8(aT, pad):  # [1024 feat, N] -> [4, 128, 2*pad] fp8
        K, N = aT.shape
        out = np.zeros((4, 2, 128, pad), np.float32)
        out[:, :, :, :N] = aT.reshape(4, 2, 128, N)
        out = out.transpose(0, 2, 1, 3).reshape(4, 128, 2 * pad)
        return np.clip(out, -240.0, 240.0).astype(F8)

    return {
        "ytd": dbl8(ys.T, 1040),
        "mtd": dbl8(Ms.T, 1040),
        "m3d": m3dd,
    }


def combine_partials(accs):
    """accs: list of 8 [128, 8] fp32 arrays -> scalar loss (float64 math)."""
    a = np.stack([np.asarray(x, np.float64) for x in accs])  # [8,128,8]
    sl_j = a[:, :, 0].sum()
    sl_m = a[:, :, 1].sum()
    sg_j = a[:, :, 2].sum()
    sg_m = a[:, :, 3].sum()
    local = BETA * (sl_m + sl_j) / (B * NI)
    glob = ALPHA * (sg_m + sg_j) / B
    return np.float32(local + glob)


def make_in_maps(inputs):
    sh = _prep_shared(inputs)
    return [dict(sh, **_prep_core(inputs, c)) for c in range(NC)]


def get_runner():
    global _RUNNER
    if _RUNNER is None:
        _RUNNER = _build_nc()
    return _RUNNER


def kernel(**inputs) -> np.ndarray:
    from concourse.bass_utils import run_bass_kernel_spmd

    nc = get_runner()
    in_maps = make_in_maps(inputs)
    res = run_bass_kernel_spmd(nc, in_maps, list(range(NC)))
    return combine_partials([r["acc"] for r in res.results])



# revision 3
# speedup vs baseline: 1.0952x; 1.0952x over previous
"""DeepInfoMax loss kernel for 8 Trainium2 NeuronCores.

Strategy (hardcoded for B=8192, d=1024, n=16):
  - Data-parallel over batch: core c gets rows [c*1024, (c+1)*1024), plus ONE
    overlap row ((c+1)*1024 % B) of M so the global roll (M_prime) is exact.
  - Activations kept feature-major ([features, batch]) on-chip; fp8 DoubleRow
    matmuls with fp32 PSUM accumulation where FD>=256.
  - net(M) (phases A/B) and the experts' y-contribution are computed once and
    shared between the joint/marginal passes.
  - Expert pipeline: the y-part psum from the grouped first-layer matmul is
    kept in PSUM; the joint M3 contribution accumulates on top (K=64 fp8 MM),
    and a single [-A;+A] K=128 MM switches the psum to the marginal pass —
    no y_part eviction or identity-plane replay.
  - Expert/global scores via PE column-tiling: masked +-w3 blocks land each
    (expert, pass) score on its own PSUM partition row, 4 col-groups running
    concurrently; softplus = exp (per-partition bias) + ln(1+x) with
    per-partition accumulation. Host sums the valid partition rows.
"""

import numpy as np
import ml_dtypes

B = 8192
D = 1024
NI = 16
DN = D // NI  # 64
NC = 8
BS = B // NC  # 1024
BSP = BS + 1  # 1025 (overlap col for the exact roll)
ALPHA = 0.5
BETA = 1.0

# column chunks over the 1025-wide (A/B) and 1024-wide phases
CH_P = [(0, 342), (342, 342), (684, 341)]
CH_C = [(0, 512), (512, 512)]

BF = ml_dtypes.bfloat16
F8 = ml_dtypes.float8_e4m3
WSC = 64.0

_RUNNER = None


def _build_nc():
    import concourse.bass as bass  # noqa: F401
    import concourse.tile as tile
    import concourse.mybir as mybir
    from concourse import bacc
    from contextlib import ExitStack

    bf = mybir.dt.bfloat16
    f32 = mybir.dt.float32
    f8 = mybir.dt.float8e4
    AF = mybir.ActivationFunctionType
    OP = mybir.AluOpType
    DR = mybir.MatmulPerfMode.DoubleRow

    nc = bacc.Bacc()

    # ---- DRAM I/O ----
    mtd = nc.dram_tensor("mtd", [4, 128, 2 * 1040], f8, kind="ExternalInput")
    ytd = nc.dram_tensor("ytd", [4, 128, 2 * 1040], f8, kind="ExternalInput")
    m3d = nc.dram_tensor("m3d", [16, 128, 1040], f8, kind="ExternalInput")
    gw0d = nc.dram_tensor("gw0d", [4, 128, 2 * D], f8, kind="ExternalInput")
    gw1d = nc.dram_tensor("gw1d", [4, 128, 2 * D], f8, kind="ExternalInput")
    bxd = nc.dram_tensor("bxd", [4, 128, 2 * 2176], f8, kind="ExternalInput")
    acatd = nc.dram_tensor("acatd", [128, 16 * 256], f8, kind="ExternalInput")
    w2sp = nc.dram_tensor("w2sp", [128, 2048], bf, kind="ExternalInput")
    l0whd = nc.dram_tensor("l0whd", [4, 128, 2 * 128], f8, kind="ExternalInput")
    l1wp = nc.dram_tensor("l1wp", [128, 128], bf, kind="ExternalInput")
    w3md = nc.dram_tensor("w3md", [128, 32 * 8], bf, kind="ExternalInput")
    w2gd = nc.dram_tensor("w2gd", [128, 4], bf, kind="ExternalInput")
    gb0c = nc.dram_tensor("gb0c", [128, 8], f32, kind="ExternalInput")
    gb1c = nc.dram_tensor("gb1c", [128, 8], f32, kind="ExternalInput")
    lb1c = nc.dram_tensor("lb1c", [128, NI], f32, kind="ExternalInput")
    lb2c = nc.dram_tensor("lb2c", [128, NI], f32, kind="ExternalInput")
    l0bc = nc.dram_tensor("l0bc", [128, 1], f32, kind="ExternalInput")
    l1bc = nc.dram_tensor("l1bc", [128, 1], f32, kind="ExternalInput")
    splb = nc.dram_tensor("splb", [128, 1], f32, kind="ExternalInput")
    spgb = nc.dram_tensor("spgb", [128, 1], f32, kind="ExternalInput")
    acc = nc.dram_tensor("acc", [128, 8], f32, kind="ExternalOutput")

    IW = 1.0 / WSC
    IW2 = 1.0 / (WSC * WSC)

    with tile.TileContext(nc) as tc, ExitStack() as ctx:
        pconst = ctx.enter_context(tc.tile_pool(name="const", bufs=1))
        pgw = ctx.enter_context(tc.tile_pool(name="gw", bufs=8))
        pi8 = ctx.enter_context(tc.tile_pool(name="i8", bufs=16))
        pbx = ctx.enter_context(tc.tile_pool(name="bx", bufs=4))
        pac = ctx.enter_context(tc.tile_pool(name="ac", bufs=1))
        pze = ctx.enter_context(tc.tile_pool(name="ze", bufs=16))
        pgy = ctx.enter_context(tc.tile_pool(name="gy", bufs=1))
        ph1 = ctx.enter_context(tc.tile_pool(name="h1", bufs=4))
        ph2 = ctx.enter_context(tc.tile_pool(name="h2", bufs=10))
        phg = ctx.enter_context(tc.tile_pool(name="hg", bufs=2))
        pex = ctx.enter_context(tc.tile_pool(name="ex", bufs=2))
        ppm = ctx.enter_context(tc.tile_pool(name="pm", bufs=5, space="PSUM"))
        pps = ctx.enter_context(tc.tile_pool(name="ps", bufs=1, space="PSUM"))

        # ---- phase A/B inputs: two HWDGE rings (sync + scalar) ----
        mt_sb = [pi8.tile([128, 2 * 1040], f8, tag="i8", name=f"mt_{k}")
                 for k in range(4)]
        for k2 in range(4):
            nc.sync.dma_start(mt_sb[k2][:], mtd[k2, :, :])
        gw0_sb = [pgw.tile([128, 2 * D], f8, tag="gw", name=f"gw0_{k}")
                  for k in range(4)]
        for k2 in range(4):
            nc.scalar.dma_start(gw0_sb[k2][:], gw0d[k2, :, :])
        gw1_sb = [pgw.tile([128, 2 * D], f8, tag="gw", name=f"gw1_{k}")
                  for k in range(4)]
        for k2 in range(4):
            nc.scalar.dma_start(gw1_sb[k2][:], gw1d[k2, :, :])

        # ---- constants (gpsimd queue) ----
        def cload(dram, shape, dt):
            t = pconst.tile(shape, dt, tag=dram.name, name=dram.name + "_sb")
            nc.gpsimd.dma_start(t[:], dram[:])
            return t

        w3m_sb = cload(w3md, [128, 32 * 8], bf)
        w2g_sb = cload(w2gd, [128, 4], bf)
        l1w_sb = cload(l1wp, [128, 128], bf)
        gb0_sb = cload(gb0c, [128, 8], f32)
        gb1_sb = cload(gb1c, [128, 8], f32)
        lb1_sb = cload(lb1c, [128, NI], f32)
        lb2_sb = cload(lb2c, [128, NI], f32)
        l0b_sb = cload(l0bc, [128, 1], f32)
        l1b_sb = cload(l1bc, [128, 1], f32)
        splb_sb = cload(splb, [128, 1], f32)
        spgb_sb = cload(spgb, [128, 1], f32)
        l0wh_sb = []
        for k2 in range(4):
            t = pconst.tile([128, 2 * 128], f8, tag=f"l0whd{k2}")
            nc.gpsimd.dma_start(t[:], l0whd[k2, :, :])
            l0wh_sb.append(t)
        acat_sb = pac.tile([128, 16 * 256], f8, tag="acat")
        nc.gpsimd.dma_start(acat_sb[:], acatd[:])
        w2s_sb = pac.tile([128, 2048], bf, tag="w2s")
        nc.gpsimd.dma_start(w2s_sb[:], w2sp[:])
        acc_sb = pconst.tile([128, 8], f32, tag="acc")
        nc.vector.memset(acc_sb[:], 0.0)

        # ze tiles: rows 0..63 joint M3_e (batch 0..1023), rows 64..127 marg
        ze_sb = []
        for e in range(16):
            t = pze.tile([128, 1040], f8, tag="ze", name=f"ze_{e}")
            nc.gpsimd.dma_start(t[:], m3d[e, :, :])
            ze_sb.append(t)

        # ---- later-phase inputs on sync ring (after mt) ----
        yt_sb = [pi8.tile([128, 2 * 1040], f8, tag="i8", name=f"yt_{k}")
                 for k in range(4)]
        for k2 in range(4):
            nc.sync.dma_start(yt_sb[k2][:], ytd[k2, :, :])
        bx_sb = [pbx.tile([128, 2 * 2176], f8, tag="bx", name=f"bx_{k}")
                 for k in range(4)]
        for k2 in range(4):
            nc.sync.dma_start(bx_sb[k2][:], bxd[k2, :, :])

        # ---- phase A: hg = WSC*relu(M@gw0+gb0), fp8 DR pairs ----
        hg_sb = [pi8.tile([128, 2 * 1040], f8, tag="i8", name=f"hg_{k}")
                 for k in range(4)]
        for m in range(8):
            for (c0, cw) in CH_P:
                ps = ppm.tile([128, 512], f32, tag="pm")
                for k2 in range(4):
                    nc.tensor.matmul(
                        ps[:, :cw],
                        gw0_sb[k2].rearrange("p (ko m) -> p ko m", ko=2)[
                            :, :, m * 128:(m + 1) * 128],
                        mt_sb[k2].rearrange("p (ko b) -> p ko b", ko=2)[
                            :, :, c0:c0 + cw],
                        start=(k2 == 0), stop=(k2 == 3), perf_mode=DR,
                    )
                nc.scalar.activation(
                    hg_sb[m // 2][:, (m % 2) * 1040 + c0:(m % 2) * 1040 + c0 + cw],
                    ps[:, :cw], AF.Relu, bias=gb0_sb[:, m:m + 1], scale=IW,
                )

        # ---- phase B: hm = WSC*(hg@gw1+gb1), fp8 DR pairs ----
        hm_sb = [pi8.tile([128, 2 * 1040], f8, tag="i8", name=f"hm_{k}")
                 for k in range(4)]
        for m in range(8):
            for (c0, cw) in CH_P:
                ps = ppm.tile([128, 512], f32, tag="pm")
                for k2 in range(4):
                    nc.tensor.matmul(
                        ps[:, :cw],
                        gw1_sb[k2].rearrange("p (ko m) -> p ko m", ko=2)[
                            :, :, m * 128:(m + 1) * 128],
                        hg_sb[k2].rearrange("p (ko b) -> p ko b", ko=2)[
                            :, :, c0:c0 + cw],
                        start=(k2 == 0), stop=(k2 == 3), perf_mode=DR,
                    )
                nc.scalar.activation(
                    hm_sb[m // 2][:, (m % 2) * 1040 + c0:(m % 2) * 1040 + c0 + cw],
                    ps[:, :cw], AF.Identity, bias=gb1_sb[:, m:m + 1], scale=IW,
                )

        # ---- gy = y@l0w[:1024] + l0b (f32, true scale) ----
        gy_sb = pgy.tile([128, BS], f32, tag="gy")
        for (c0, cw) in CH_C:
            ps = ppm.tile([128, 512], f32, tag="pm")
            for k2 in range(4):
                nc.tensor.matmul(
                    ps[:, :cw],
                    bx_sb[k2].rearrange("p (ko m) -> p ko m", ko=2)[
                        :, :, 16 * 128:17 * 128],
                    yt_sb[k2].rearrange("p (ko b) -> p ko b", ko=2)[
                        :, :, c0:c0 + cw],
                    start=(k2 == 0), stop=(k2 == 3), perf_mode=DR,
                )
            nc.scalar.activation(
                gy_sb[:, c0:c0 + cw], ps[:, :cw], AF.Identity,
                bias=l0b_sb[:, 0:1], scale=IW2,
            )

        # ---- phase F: global discriminator, col-tiled l2 scores ----
        psg = pps.tile([128, 512], f32, tag="pg", name="psum_g")
        for p in range(2):
            for ci, (c0, cw) in enumerate(CH_C):
                ps = ppm.tile([128, 512], f32, tag="pm")
                for k2 in range(4):
                    nc.tensor.matmul(
                        ps[:, :cw],
                        l0wh_sb[k2].rearrange("p (ko m) -> p ko m", ko=2),
                        hm_sb[k2].rearrange("p (ko b) -> p ko b", ko=2)[
                            :, :, p + c0:p + c0 + cw],
                        start=(k2 == 0), stop=(k2 == 3), perf_mode=DR,
                    )
                z0 = phg.tile([128, 512], bf, tag="z0")
                nc.vector.scalar_tensor_tensor(
                    z0[:, :cw], ps[:, :cw], IW2, gy_sb[:, c0:c0 + cw],
                    op0=OP.mult, op1=OP.add)
                h0 = phg.tile([128, 512], bf, tag="h0")
                nc.scalar.activation(h0[:, :cw], z0[:, :cw], AF.Relu)
                ps1 = ppm.tile([128, 512], f32, tag="pm")
                nc.tensor.matmul(
                    ps1[:, :cw], l1w_sb[:], h0[:, :cw], start=True, stop=True)
                h1g = phg.tile([128, 512], bf, tag="h1g")
                nc.scalar.activation(
                    h1g[:, :cw], ps1[:, :cw], AF.Relu, bias=l1b_sb[:, 0:1])
                j = 2 * ci + p
                nc.tensor.matmul(
                    psg[32 * j:32 * j + 1, :cw],
                    w2g_sb[:, j:j + 1],
                    h1g[:, :cw],
                    start=True, stop=True,
                    tile_position=(0, 32 * j), skip_group_check=True,
                )
        exg = pex.tile([128, 512], f32, tag="ex", name="exg")
        nc.scalar.activation(exg[:], psg[:], AF.Exp, bias=spgb_sb[:, 0:1])
        spg_t = pex.tile([128, 512], f32, tag="ex", name="spg")
        nc.scalar.activation(spg_t[:], exg[:], AF.Ln, bias=1.0,
                             accum_out=acc_sb[:, 2:3])

        # ---- expert loop: merged y-part + joint/marg z1, L2, batched L3 ----
        ps_loc = [pps.tile([128, 512], f32, tag=f"S{ci}", name=f"S_{ci}")
                  for ci in range(2)]
        h2_tiles = {}
        for e in range(NI):
            h1t = {p: ph1.tile([128, BS], bf, tag="h1", name=f"h1_{e}_{p}")
                   for p in range(2)}
            for ci, (c0, cw) in enumerate(CH_C):
                ps = ppm.tile([128, 512], f32, tag="pm")
                for k2 in range(4):
                    nc.tensor.matmul(
                        ps[:, :cw],
                        bx_sb[k2].rearrange("p (ko m) -> p ko m", ko=2)[
                            :, :, e * 128:(e + 1) * 128],
                        yt_sb[k2].rearrange("p (ko b) -> p ko b", ko=2)[
                            :, :, c0:c0 + cw],
                        start=(k2 == 0), stop=(k2 == 3), perf_mode=DR,
                    )
                # joint: += A_e @ M3_joint (rows 0..63 of ze)
                nc.tensor.matmul(
                    ps[:, :cw],
                    acat_sb[0:64, e * 256:e * 256 + 128],
                    ze_sb[e][0:64, c0:c0 + cw],
                    start=False, stop=False, skip_group_check=True,
                )
                nc.scalar.activation(
                    h1t[0][:, c0:c0 + cw], ps[:, :cw], AF.Relu,
                    bias=lb1_sb[:, e:e + 1], scale=IW2)
                # switch to marg: += [-A;+A] @ [M3_joint; M3_marg]
                nc.tensor.matmul(
                    ps[:, :cw],
                    acat_sb[:, e * 256 + 128:e * 256 + 256],
                    ze_sb[e][:, c0:c0 + cw],
                    start=False, stop=True, skip_group_check=True,
                )
                nc.scalar.activation(
                    h1t[1][:, c0:c0 + cw], ps[:, :cw], AF.Relu,
                    bias=lb1_sb[:, e:e + 1], scale=IW2)
            for p in range(2):
                h2t = ph2.tile([128, BS], bf, tag="h2", name=f"h2_{e}_{p}")
                h2_tiles[(e, p)] = h2t
                for ci, (c0, cw) in enumerate(CH_C):
                    ps2 = ppm.tile([128, 512], f32, tag="pm")
                    nc.tensor.matmul(
                        ps2[:, :cw],
                        w2s_sb[:, e * 128:(e + 1) * 128],
                        h1t[p][:, c0:c0 + cw],
                        start=True, stop=True,
                    )
                    nc.vector.tensor_scalar(
                        h2t[:, c0:c0 + cw], ps2[:, :cw],
                        lb2_sb[:, e:e + 1], 0.0, op0=OP.add, op1=OP.max)
            # L3 burst every 4 experts: 4-way col-tiled concurrent MMs
            if e % 4 == 3:
                t = e // 4
                for ci, (c0, cw) in enumerate(CH_C):
                    for p in range(2):
                        for j in range(4):
                            eb = 4 * t + j
                            blk = eb * 2 + p
                            nc.tensor.matmul(
                                ps_loc[ci][32 * j:32 * j + 8, :cw],
                                w3m_sb[:, blk * 8:(blk + 1) * 8],
                                h2_tiles[(eb, p)][:, c0:c0 + cw],
                                start=(t == 0 and p == 0),
                                stop=(t == 3 and p == 1),
                                tile_position=(0, 32 * j),
                                skip_group_check=True,
                            )
                if t == 3:
                    for ci in range(2):
                        exl = pex.tile([128, 512], f32, tag="ex",
                                       name=f"exl{ci}")
                        nc.scalar.activation(
                            exl[:], ps_loc[ci][:], AF.Exp,
                            bias=splb_sb[:, 0:1])
                        spl_t = pex.tile([128, 512], f32, tag="ex",
                                         name=f"spl{ci}")
                        nc.scalar.activation(
                            spl_t[:], exl[:], AF.Ln, bias=1.0,
                            accum_out=acc_sb[:, ci:ci + 1])

        # ---- output ----
        nc.sync.dma_start(acc[:], acc_sb[:])

    nc.finalize()
    return nc


def _prep_shared(inputs):
    """Weight repack (identical for all cores)."""
    f32 = np.float32
    gw0 = np.asarray(inputs["gw0"], f32)
    gw1 = np.asarray(inputs["gw1"], f32)
    l0w = np.asarray(inputs["l0w"], f32)
    l1w = np.asarray(inputs["l1w"], f32)
    l2w = np.asarray(inputs["l2w"], f32)
    lW1 = np.asarray(inputs["lW1"], f32)
    lW2 = np.asarray(inputs["lW2"], f32)
    lW3 = np.asarray(inputs["lW3"], f32)
    gb0 = np.asarray(inputs["gb0"], f32)
    gb1 = np.asarray(inputs["gb1"], f32)
    l0b = np.asarray(inputs["l0b"], f32)
    l1b = np.asarray(inputs["l1b"], f32)
    l2b = np.asarray(inputs["l2b"], f32)
    lb1 = np.asarray(inputs["lb1"], f32)
    lb2 = np.asarray(inputs["lb2"], f32)
    lb3 = np.asarray(inputs["lb3"], f32)

    def dbl(a, scale=1.0, pad=None):
        # [1024, N] -> [4, 128, 2*Np] fp8 DoubleRow: f = k2*256 + ko*128 + ki
        K, N = a.shape
        Np = N if pad is None else pad
        out = np.zeros((4, 2, 128, Np), np.float32)
        out[:, :, :, :N] = a.reshape(4, 2, 128, N) * scale
        out = out.transpose(0, 2, 1, 3).reshape(4, 128, 2 * Np)
        return np.clip(out, -240.0, 240.0).astype(F8)

    # acatd: per e, block0 rows 0..63 = WSC*A_e; block1 = [-WSC*A; +WSC*A]
    acat = np.zeros((128, NI * 256), np.float32)
    for e in range(NI):
        A = lW1[e, :DN, :] * WSC  # [64, 128]
        acat[:DN, e * 256:e * 256 + 128] = A
        acat[:DN, e * 256 + 128:e * 256 + 256] = -A
        acat[DN:, e * 256 + 128:e * 256 + 256] = A
    acat = np.clip(acat, -240, 240).astype(F8)

    # w3md: block (e, p) = [128, 8], col (4p + e//4) = sign(p)*w3_e
    w3m = np.zeros((128, 32 * 8), f32)
    for e in range(NI):
        for p in range(2):
            blk = e * 2 + p
            s = 4 * p + e // 4
            sgn = -1.0 if p == 0 else 1.0
            w3m[:, blk * 8 + s] = sgn * lW3[e, :, 0]

    # w2gd: col j = 2ci+p -> sign(p)*l2w
    w2g = np.zeros((128, 4), f32)
    for ci in range(2):
        for p in range(2):
            sgn = -1.0 if p == 0 else 1.0
            w2g[:, 2 * ci + p] = sgn * l2w[:, 0]

    # softplus bias rows: local r = 32j+4p+t (e=4t+j): -+lb3[e]
    splb_ = np.zeros((128, 1), f32)
    for e in range(NI):
        j, t = e % 4, e // 4
        for p in range(2):
            sgn = -1.0 if p == 0 else 1.0
            splb_[32 * j + 4 * p + t, 0] = sgn * lb3[e, 0]
    # global r = 32j (j=2ci+p): -+l2b
    spgb_ = np.zeros((128, 1), f32)
    for ci in range(2):
        for p in range(2):
            sgn = -1.0 if p == 0 else 1.0
            spgb_[32 * (2 * ci + p), 0] = sgn * l2b[0]

    # l0whd: l0w[1024:2048] DR repack, WSC-scaled
    l0wh = l0w[D:].reshape(4, 2, 128, 128) * WSC
    l0wh = np.clip(l0wh.transpose(0, 2, 1, 3).reshape(4, 128, 256),
                   -240, 240).astype(F8)

    bcatx = np.concatenate(
        [lW1[:, DN:, :].transpose(1, 0, 2).reshape(D, NI * 128), l0w[:D]],
        axis=1)
    sh = {
        "gw0d": dbl(gw0, WSC),
        "gw1d": dbl(gw1, WSC),
        "bxd": dbl(bcatx, WSC, pad=2176),
        "acatd": acat,
        "w2sp": np.ascontiguousarray(
            lW2.transpose(1, 0, 2).reshape(128, NI * 128)).astype(BF),
        "l0whd": l0wh,
        "l1wp": l1w.astype(BF),
        "w3md": w3m.astype(BF),
        "w2gd": w2g.astype(BF),
        "gb0c": np.ascontiguousarray(gb0.reshape(8, 128).T) * WSC,
        "gb1c": np.ascontiguousarray(gb1.reshape(8, 128).T) * WSC,
        "lb1c": np.ascontiguousarray(lb1.T),
        "lb2c": np.ascontiguousarray(lb2.T),
        "l0bc": np.ascontiguousarray(l0b[:, None]),
        "l1bc": np.ascontiguousarray(l1b[:, None]),
        "splb": splb_,
        "spgb": spgb_,
    }
    return sh


def _prep_core(inputs, c):
    f32 = np.float32
    y = np.asarray(inputs["y"], f32)
    M = np.asarray(inputs["M"], f32)
    r0 = c * BS
    rows = np.arange(r0, r0 + BSP) % B  # 1025 rows incl. overlap
    Ms = M[rows]  # [1025, 1024]
    ys = y[r0:r0 + BS]  # [1024, 1024]
    # expert-major M: m3t[e, p, b] = Ms[b, p*16+e]
    m3t = np.ascontiguousarray(
        Ms.reshape(BSP, DN, NI).transpose(2, 1, 0))  # [16,64,1025] f32

    # m3d[e]: [128, 1040]: rows 0..63 joint (b 0..1023), 64..127 marg (b 1..)
    m3dd = np.zeros((NI, 128, 1040), np.float32)
    m3dd[:, :DN, 0:BS] = m3t[:, :, 0:BS] * WSC
    m3dd[:, DN:, 0:BS] = m3t[:, :, 1:BS + 1] * WSC
    m3dd = np.clip(m3dd, -240, 240).astype(F8)

    def dbl8(aT, pad):  # [1024 feat, N] -> [4, 128, 2*pad] fp8
        K, N = aT.shape
        out = np.zeros((4, 2, 128, pad), np.float32)
        out[:, :, :, :N] = aT.reshape(4, 2, 128, N) * WSC
        out = out.transpose(0, 2, 1, 3).reshape(4, 128, 2 * pad)
        return np.clip(out, -240.0, 240.0).astype(F8)

    return {
        "ytd": dbl8(ys.T, 1040),
        "mtd": dbl8(Ms.T, 1040),
        "m3d": m3dd,
    }


_LMASK = np.zeros(128, bool)
_GMASK = np.zeros(128, bool)
for _e in range(NI):
    for _p in range(2):
        _LMASK[32 * (_e % 4) + 4 * _p + _e // 4] = True
for _j in range(4):
    _GMASK[32 * _j] = True


def combine_partials(accs):
    """accs: list of 8 [128, 8] fp32 arrays -> scalar loss (float64 math)."""
    a = np.stack([np.asarray(x, np.float64) for x in accs])  # [8,128,8]
    local = a[:, _LMASK, 0:2].sum()
    glob = a[:, _GMASK, 2].sum()
    return np.float32(BETA * local / (B * NI) + ALPHA * glob / B)


def make_in_maps(inputs):
    sh = _prep_shared(inputs)
    return [dict(sh, **_prep_core(inputs, c)) for c in range(NC)]


def get_runner():
    global _RUNNER
    if _RUNNER is None:
        _RUNNER = _build_nc()
    return _RUNNER


def kernel(**inputs) -> np.ndarray:
    from concourse.bass_utils import run_bass_kernel_spmd

    nc = get_runner()
    in_maps = make_in_maps(inputs)
    res = run_bass_kernel_spmd(nc, in_maps, list(range(NC)))
    return combine_partials([r["acc"] for r in res.results])


# revision 7
# speedup vs baseline: 1.2817x; 1.1703x over previous
"""DeepInfoMax loss kernel for 8 Trainium2 NeuronCores.

Strategy (hardcoded for B=8192, d=1024, n=16):
  - Data-parallel over batch: core c gets rows [c*1024, (c+1)*1024), plus ONE
    overlap row ((c+1)*1024 % B) of M so the global roll (M_prime) is exact.
  - Activations kept feature-major ([features, batch]) on-chip; fp8 DoubleRow
    matmuls with fp32 PSUM accumulation where FD>=256.
  - net(M) (phases A/B) and the experts' y-contribution are computed once and
    shared between the joint/marginal passes.
  - Expert pipeline: the y-part psum from the grouped first-layer matmul is
    kept in PSUM; the joint M3 contribution accumulates on top (K=64 fp8 MM),
    and a single [-A;+A] K=128 MM switches the psum to the marginal pass.
    Software-pipelined: expert e-1's second L2 pass fills the PE while expert
    e's h1 evictions run, hiding the psum WAR stalls.
  - All 36 scores (32 expert + 4 global) land on separate partition rows of
    two PSUM banks via PE column-tiling with masked +-w columns; one Softplus
    activation per bank (per-partition bias, accumulate) finishes the loss.
    Host sums the valid partition rows.
"""

import numpy as np
import ml_dtypes

B = 8192
D = 1024
NI = 16
DN = D // NI  # 64
NC = 8
BS = B // NC  # 1024
BSP = BS + 1  # 1025 (overlap col for the exact roll)
ALPHA = 0.5
BETA = 1.0

CH_P = [(0, 342), (342, 342), (684, 341)]
CH_C = [(0, 512), (512, 512)]

BF = ml_dtypes.bfloat16
F8 = ml_dtypes.float8_e4m3
WSC = 64.0

_RUNNER = None

# cstd column map (f32 consts)
C_GB0 = 0      # 8 cols: WSC*gb0 per m-tile
C_GB1 = 8      # 8 cols: WSC^2*gb1 per m-tile
C_LB1 = 16     # 16 cols: lb1 per expert
C_LB2 = 32     # 16 cols: lb2 per expert
C_L0B = 48
C_L1B = 49
C_SPB = 50     # softplus bias rows (local + global)
C_ZERO = 51
NCST = 52

# cbfd column map (bf16 consts)
B_W3M = 0      # 32 blocks x 8 cols: masked +-w3
B_W2G = 256    # 2 blocks x 9 cols: masked +-l2w (col 8 of each block)
B_L1W = 274    # 128 cols: l1w
NBF = 402


def _build_nc():
    import concourse.bass as bass  # noqa: F401
    import concourse.tile as tile
    import concourse.mybir as mybir
    from concourse import bacc
    from contextlib import ExitStack

    bf = mybir.dt.bfloat16
    f32 = mybir.dt.float32
    f8 = mybir.dt.float8e4
    AF = mybir.ActivationFunctionType
    OP = mybir.AluOpType
    DR = mybir.MatmulPerfMode.DoubleRow

    nc = bacc.Bacc()

    mtd = nc.dram_tensor("mtd", [4, 128, 2 * 1040], f8, kind="ExternalInput")
    ytd = nc.dram_tensor("ytd", [4, 128, 2 * 1040], f8, kind="ExternalInput")
    m3d = nc.dram_tensor("m3d", [128, 16 * 1040], f8, kind="ExternalInput")
    gw0d = nc.dram_tensor("gw0d", [4, 128, 2 * D], f8, kind="ExternalInput")
    gw1d = nc.dram_tensor("gw1d", [4, 128, 2 * D], f8, kind="ExternalInput")
    bxd = nc.dram_tensor("bxd", [4, 128, 2 * 2176], f8, kind="ExternalInput")
    acatd = nc.dram_tensor("acatd", [128, 16 * 256], f8, kind="ExternalInput")
    w2sp = nc.dram_tensor("w2sp", [128, 2048], bf, kind="ExternalInput")
    l0whd = nc.dram_tensor("l0whd", [128, 4 * 256], f8, kind="ExternalInput")
    cstd = nc.dram_tensor("cstd", [128, NCST], f32, kind="ExternalInput")
    cbfd = nc.dram_tensor("cbfd", [128, NBF], bf, kind="ExternalInput")
    acc = nc.dram_tensor("acc", [128, 8], f32, kind="ExternalOutput")

    IW = 1.0 / WSC
    IW2 = 1.0 / (WSC * WSC)

    with tile.TileContext(nc) as tc, ExitStack() as ctx:
        pconst = ctx.enter_context(tc.tile_pool(name="const", bufs=1))
        pgw = ctx.enter_context(tc.tile_pool(name="gw", bufs=8))
        pi8 = ctx.enter_context(tc.tile_pool(name="i8", bufs=16))
        pbx = ctx.enter_context(tc.tile_pool(name="bx", bufs=4))
        pac = ctx.enter_context(tc.tile_pool(name="ac", bufs=1))
        pze = ctx.enter_context(tc.tile_pool(name="ze", bufs=1))
        ph1 = ctx.enter_context(tc.tile_pool(name="h1", bufs=6))
        ph2 = ctx.enter_context(tc.tile_pool(name="h2", bufs=12))
        phg = ctx.enter_context(tc.tile_pool(name="hg", bufs=2))
        pex = ctx.enter_context(tc.tile_pool(name="ex", bufs=2))
        ppm = ctx.enter_context(tc.tile_pool(name="pm", bufs=6, space="PSUM"))
        pps = ctx.enter_context(tc.tile_pool(name="ps", bufs=1, space="PSUM"))

        # ---- phase A inputs on the two HWDGE rings ----
        mt_sb = [pi8.tile([128, 2 * 1040], f8, tag="i8", name=f"mt_{k}")
                 for k in range(4)]
        for k2 in range(4):
            nc.sync.dma_start(mt_sb[k2][:], mtd[k2, :, :])
        gw0_sb = [pgw.tile([128, 2 * D], f8, tag="gw", name=f"gw0_{k}")
                  for k in range(4)]
        for k2 in range(4):
            nc.scalar.dma_start(gw0_sb[k2][:], gw0d[k2, :, :])
        gw1_sb = [pgw.tile([128, 2 * D], f8, tag="gw", name=f"gw1_{k}")
                  for k in range(4)]
        for k2 in range(4):
            nc.scalar.dma_start(gw1_sb[k2][:], gw1d[k2, :, :])

        # ---- consts (gpsimd queue) ----
        cst = pconst.tile([128, NCST], f32, tag="cst")
        nc.gpsimd.dma_start(cst[:], cstd[:])
        cbf = pconst.tile([128, NBF], bf, tag="cbf")
        nc.gpsimd.dma_start(cbf[:], cbfd[:])
        l0wh_sb = pconst.tile([128, 4 * 256], f8, tag="l0wh")
        nc.gpsimd.dma_start(l0wh_sb[:], l0whd[:])
        acat_sb = pac.tile([128, 16 * 256], f8, tag="acat")
        nc.gpsimd.dma_start(acat_sb[:], acatd[:])
        w2s_sb = pac.tile([128, 2048], bf, tag="w2s")
        nc.gpsimd.dma_start(w2s_sb[:], w2sp[:])
        ze_sb = pze.tile([128, 16 * 1040], f8, tag="ze")
        nc.gpsimd.dma_start(ze_sb[:], m3d[:])
        acc_sb = pconst.tile([128, 8], f32, tag="acc")
        nc.vector.memset(acc_sb[:], 0.0)
        scr = pconst.tile([128, 1], f32, tag="scr")
        # dummy exp+ln: pull in the natural_log_exp table set before any real
        # ACT work so the tail softplus pays no table switch
        nc.scalar.activation(scr[:], cst[:, C_ZERO:C_ZERO + 1], AF.Exp)
        nc.scalar.activation(scr[:], cst[:, C_ZERO:C_ZERO + 1], AF.Ln,
                             bias=1.0)

        # ---- later-phase inputs on sync ring (no buf reuse -> no waits) ----
        yt_sb = [pi8.tile([128, 2 * 1040], f8, tag="i8", name=f"yt_{k}")
                 for k in range(4)]
        for k2 in range(4):
            nc.sync.dma_start(yt_sb[k2][:], ytd[k2, :, :])
        bx_sb = [pbx.tile([128, 2 * 2176], f8, tag="bx", name=f"bx_{k}")
                 for k in range(4)]
        for k2 in range(4):
            nc.sync.dma_start(bx_sb[k2][:], bxd[k2, :, :])

        # ---- phase A: hg = WSC*relu(M@gw0+gb0), fp8 DR pairs (ACT evict) ----
        hg_sb = [pi8.tile([128, 2 * 1040], f8, tag="i8", name=f"hg_{k}")
                 for k in range(4)]
        for m in range(8):
            for (c0, cw) in CH_P:
                ps = ppm.tile([128, 512], f32, tag="pm")
                for k2 in range(4):
                    nc.tensor.matmul(
                        ps[:, :cw],
                        gw0_sb[k2].rearrange("p (ko m) -> p ko m", ko=2)[
                            :, :, m * 128:(m + 1) * 128],
                        mt_sb[k2].rearrange("p (ko b) -> p ko b", ko=2)[
                            :, :, c0:c0 + cw],
                        start=(k2 == 0), stop=(k2 == 3), perf_mode=DR,
                    )
                nc.scalar.activation(
                    hg_sb[m // 2][:, (m % 2) * 1040 + c0:(m % 2) * 1040 + c0 + cw],
                    ps[:, :cw], AF.Relu,
                    bias=cst[:, C_GB0 + m:C_GB0 + m + 1], scale=IW,
                )

        # ---- phase B: hm = WSC*(hg@gw1+gb1), fp8 DR pairs (DVE evict) ----
        hm_sb = [pi8.tile([128, 2 * 1040], f8, tag="i8", name=f"hm_{k}")
                 for k in range(4)]
        for m in range(8):
            for (c0, cw) in CH_P:
                ps = ppm.tile([128, 512], f32, tag="pm")
                for k2 in range(4):
                    nc.tensor.matmul(
                        ps[:, :cw],
                        gw1_sb[k2].rearrange("p (ko m) -> p ko m", ko=2)[
                            :, :, m * 128:(m + 1) * 128],
                        hg_sb[k2].rearrange("p (ko b) -> p ko b", ko=2)[
                            :, :, c0:c0 + cw],
                        start=(k2 == 0), stop=(k2 == 3), perf_mode=DR,
                    )
                nc.vector.tensor_scalar(
                    hm_sb[m // 2][:, (m % 2) * 1040 + c0:(m % 2) * 1040 + c0 + cw],
                    ps[:, :cw], cst[:, C_GB1 + m:C_GB1 + m + 1], IW,
                    op0=OP.add, op1=OP.mult,
                )

        # score psum banks: rows 32j+4p+t = expert (e=4t+j, pass p);
        # rows 32p+8 = global pass p. One per batch-column chunk.
        ps_loc = [pps.tile([128, 512], f32, tag=f"S{ci}", name=f"S_{ci}")
                  for ci in range(2)]

        # ---- phase F: global discriminator (gy folded into psum),
        #      software-pipelined depth 2 ----
        FIT = [(p, ci) for p in range(2) for ci in range(2)]
        fh0 = [None] * 4
        fh1g = [None] * 4
        for it in range(6):
            if it < 4:
                p, ci = FIT[it]
                c0, cw = CH_C[ci]
                ps = ppm.tile([128, 512], f32, tag="pm")
                for k2 in range(4):
                    nc.tensor.matmul(
                        ps[:, :cw],
                        bx_sb[k2].rearrange("p (ko m) -> p ko m", ko=2)[
                            :, :, 16 * 128:17 * 128],
                        yt_sb[k2].rearrange("p (ko b) -> p ko b", ko=2)[
                            :, :, c0:c0 + cw],
                        start=(k2 == 0), stop=False, perf_mode=DR,
                    )
                for k2 in range(4):
                    nc.tensor.matmul(
                        ps[:, :cw],
                        l0wh_sb[:, k2 * 256:(k2 + 1) * 256].rearrange(
                            "p (ko m) -> p ko m", ko=2),
                        hm_sb[k2].rearrange("p (ko b) -> p ko b", ko=2)[
                            :, :, p + c0:p + c0 + cw],
                        start=False, stop=(k2 == 3), perf_mode=DR,
                    )
                h0 = phg.tile([128, 512], bf, tag="h0")
                nc.scalar.activation(
                    h0[:, :cw], ps[:, :cw], AF.Relu,
                    bias=cst[:, C_L0B:C_L0B + 1], scale=IW2)
                fh0[it] = h0
            if 1 <= it <= 4:
                pp_, ci_ = FIT[it - 1]
                c0, cw = CH_C[ci_]
                ps1 = ppm.tile([128, 512], f32, tag="pm")
                nc.tensor.matmul(
                    ps1[:, :cw], cbf[:, B_L1W:B_L1W + 128],
                    fh0[it - 1][:, :cw], start=True, stop=True)
                h1g = phg.tile([128, 512], bf, tag="h1g")
                nc.scalar.activation(
                    h1g[:, :cw], ps1[:, :cw], AF.Relu,
                    bias=cst[:, C_L1B:C_L1B + 1])
                fh1g[it - 1] = h1g
            if 2 <= it:
                pp_, ci_ = FIT[it - 2]
                c0, cw = CH_C[ci_]
                nc.tensor.matmul(
                    ps_loc[ci_][32 * pp_:32 * pp_ + 9, :cw],
                    cbf[:, B_W2G + pp_ * 9:B_W2G + (pp_ + 1) * 9],
                    fh1g[it - 2][:, :cw],
                    start=True, stop=True,
                    tile_position=(0, 32 * pp_), skip_group_check=True,
                )

        # ---- expert loop, software-pipelined ----
        h1_all = {}
        h2_tiles = {}

        def emit_L2(e, p):
            for ci, (c0, cw) in enumerate(CH_C):
                ps2 = ppm.tile([128, 512], f32, tag="pm")
                nc.tensor.matmul(
                    ps2[:, :cw],
                    w2s_sb[:, e * 128:(e + 1) * 128],
                    h1_all[(e, p)][:, c0:c0 + cw],
                    start=True, stop=True,
                )
                h2t = h2_tiles[(e, p)]
                nc.vector.tensor_scalar(
                    h2t[:, c0:c0 + cw], ps2[:, :cw],
                    cst[:, C_LB2 + e:C_LB2 + e + 1], 0.0,
                    op0=OP.add, op1=OP.max)

        def emit_burst(t):
            for ci, (c0, cw) in enumerate(CH_C):
                for p in range(2):
                    for j in range(4):
                        eb = 4 * t + j
                        blk = eb * 2 + p
                        nc.tensor.matmul(
                            ps_loc[ci][32 * j:32 * j + 8, :cw],
                            cbf[:, B_W3M + blk * 8:B_W3M + (blk + 1) * 8],
                            h2_tiles[(eb, p)][:, c0:c0 + cw],
                            # strips j>=2 see no F score MM; their first
                            # writer must reset the strip's has_written bits
                            start=(t == 0 and p == 0 and j >= 2),
                            stop=(t == 3 and p == 1),
                            tile_position=(0, 32 * j),
                            skip_group_check=True,
                        )

        for e in range(NI):
            for p in range(2):
                h1_all[(e, p)] = ph1.tile([128, BS], bf, tag="h1",
                                          name=f"h1_{e}_{p}")
                h2_tiles[(e, p)] = ph2.tile([128, BS], bf, tag="h2",
                                            name=f"h2_{e}_{p}")
            psC = []
            # stage 1: y-part + joint M3 into psum, evict h1 pass 0
            for ci, (c0, cw) in enumerate(CH_C):
                ps = ppm.tile([128, 512], f32, tag="pm")
                psC.append(ps)
                for k2 in range(4):
                    nc.tensor.matmul(
                        ps[:, :cw],
                        bx_sb[k2].rearrange("p (ko m) -> p ko m", ko=2)[
                            :, :, e * 128:(e + 1) * 128],
                        yt_sb[k2].rearrange("p (ko b) -> p ko b", ko=2)[
                            :, :, c0:c0 + cw],
                        start=(k2 == 0), stop=(k2 == 3), perf_mode=DR,
                    )
                nc.tensor.matmul(
                    ps[:, :cw],
                    acat_sb[0:64, e * 256:e * 256 + 128],
                    ze_sb[0:64, e * 1040 + c0:e * 1040 + c0 + cw],
                    start=False, stop=False, skip_group_check=True,
                )
                nc.scalar.activation(
                    h1_all[(e, 0)][:, c0:c0 + cw], ps[:, :cw], AF.Relu,
                    bias=cst[:, C_LB1 + e:C_LB1 + e + 1], scale=IW2)
            # deferred L2 pass 1 of previous expert fills the PE while the
            # h1 evictions above drain
            if e >= 1:
                emit_L2(e - 1, 1)
            # stage 2: switch psum to the marginal pass, evict h1 pass 1
            for ci, (c0, cw) in enumerate(CH_C):
                nc.tensor.matmul(
                    psC[ci][:, :cw],
                    acat_sb[:, e * 256 + 128:e * 256 + 256],
                    ze_sb[:, e * 1040 + c0:e * 1040 + c0 + cw],
                    start=False, stop=True, skip_group_check=True,
                )
                nc.scalar.activation(
                    h1_all[(e, 1)][:, c0:c0 + cw], psC[ci][:, :cw], AF.Relu,
                    bias=cst[:, C_LB1 + e:C_LB1 + e + 1], scale=IW2)
            emit_L2(e, 0)
            if e % 4 == 0 and e >= 4:
                emit_burst(e // 4 - 1)

        emit_L2(NI - 1, 1)
        # final burst + softplus reduction, pipelined by chunk
        t = 3
        for ci, (c0, cw) in enumerate(CH_C):
            for p in range(2):
                for j in range(4):
                    eb = 4 * t + j
                    blk = eb * 2 + p
                    nc.tensor.matmul(
                        ps_loc[ci][32 * j:32 * j + 8, :cw],
                        cbf[:, B_W3M + blk * 8:B_W3M + (blk + 1) * 8],
                        h2_tiles[(eb, p)][:, c0:c0 + cw],
                        start=False, stop=(p == 1),
                        tile_position=(0, 32 * j),
                        skip_group_check=True,
                    )
            ex_t = pex.tile([128, 512], f32, tag="ex", name=f"ex{ci}")
            nc.scalar.activation(
                ex_t[:], ps_loc[ci][:], AF.Exp,
                bias=cst[:, C_SPB:C_SPB + 1])
            spl = pex.tile([128, 512], f32, tag="ex", name=f"spl{ci}")
            nc.scalar.activation(
                spl[:], ex_t[:], AF.Ln, bias=1.0,
                accum_out=acc_sb[:, ci:ci + 1])

        nc.sync.dma_start(acc[:], acc_sb[:])

    nc.finalize()
    return nc


def _prep_shared(inputs):
    """Weight repack (identical for all cores)."""
    f32 = np.float32
    gw0 = np.asarray(inputs["gw0"], f32)
    gw1 = np.asarray(inputs["gw1"], f32)
    l0w = np.asarray(inputs["l0w"], f32)
    l1w = np.asarray(inputs["l1w"], f32)
    l2w = np.asarray(inputs["l2w"], f32)
    lW1 = np.asarray(inputs["lW1"], f32)
    lW2 = np.asarray(inputs["lW2"], f32)
    lW3 = np.asarray(inputs["lW3"], f32)
    gb0 = np.asarray(inputs["gb0"], f32)
    gb1 = np.asarray(inputs["gb1"], f32)
    l0b = np.asarray(inputs["l0b"], f32)
    l1b = np.asarray(inputs["l1b"], f32)
    l2b = np.asarray(inputs["l2b"], f32)
    lb1 = np.asarray(inputs["lb1"], f32)
    lb2 = np.asarray(inputs["lb2"], f32)
    lb3 = np.asarray(inputs["lb3"], f32)

    def dbl(a, scale=1.0, pad=None):
        K, N = a.shape
        Np = N if pad is None else pad
        out = np.zeros((4, 2, 128, Np), np.float32)
        out[:, :, :, :N] = a.reshape(4, 2, 128, N) * scale
        out = out.transpose(0, 2, 1, 3).reshape(4, 128, 2 * Np)
        return np.clip(out, -240.0, 240.0).astype(F8)

    acat = np.zeros((128, NI * 256), np.float32)
    for e in range(NI):
        A = lW1[e, :DN, :] * WSC
        acat[:DN, e * 256:e * 256 + 128] = A
        acat[:DN, e * 256 + 128:e * 256 + 256] = -A
        acat[DN:, e * 256 + 128:e * 256 + 256] = A
    acat = np.clip(acat, -240, 240).astype(F8)

    cbf = np.zeros((128, NBF), f32)
    for e in range(NI):
        for p in range(2):
            blk = e * 2 + p
            s = 4 * p + e // 4
            sgn = -1.0 if p == 0 else 1.0
            cbf[:, B_W3M + blk * 8 + s] = sgn * lW3[e, :, 0]
    for p in range(2):
        sgn = -1.0 if p == 0 else 1.0
        cbf[:, B_W2G + p * 9 + 8] = sgn * l2w[:, 0]
    cbf[:, B_L1W:B_L1W + 128] = l1w

    cst = np.zeros((128, NCST), f32)
    cst[:, C_GB0:C_GB0 + 8] = gb0.reshape(8, 128).T * WSC
    cst[:, C_GB1:C_GB1 + 8] = gb1.reshape(8, 128).T * (WSC * WSC)
    cst[:, C_LB1:C_LB1 + NI] = lb1.T
    cst[:, C_LB2:C_LB2 + NI] = lb2.T
    for e in range(NI):
        j, t = e % 4, e // 4
        for p in range(2):
            sgn = -1.0 if p == 0 else 1.0
            cst[32 * j + 4 * p + t, C_SPB] = sgn * lb3[e, 0]
    for p in range(2):
        sgn = -1.0 if p == 0 else 1.0
        cst[32 * p + 8, C_SPB] = sgn * l2b[0]
    cst[:, C_L0B] = l0b
    cst[:, C_L1B] = l1b

    l0wh = l0w[D:].reshape(4, 2, 128, 128) * WSC
    l0wh = np.clip(l0wh.transpose(2, 0, 1, 3).reshape(128, 4 * 256),
                   -240, 240).astype(F8)

    bcatx = np.concatenate(
        [lW1[:, DN:, :].transpose(1, 0, 2).reshape(D, NI * 128), l0w[:D]],
        axis=1)
    return {
        "gw0d": dbl(gw0, WSC),
        "gw1d": dbl(gw1, WSC),
        "bxd": dbl(bcatx, WSC, pad=2176),
        "acatd": acat,
        "w2sp": np.ascontiguousarray(
            lW2.transpose(1, 0, 2).reshape(128, NI * 128)).astype(BF),
        "l0whd": l0wh,
        "cstd": cst,
        "cbfd": cbf.astype(BF),
    }


def _prep_core(inputs, c):
    f32 = np.float32
    y = np.asarray(inputs["y"], f32)
    M = np.asarray(inputs["M"], f32)
    r0 = c * BS
    rows = np.arange(r0, r0 + BSP) % B
    Ms = M[rows]
    ys = y[r0:r0 + BS]
    m3t = np.ascontiguousarray(
        Ms.reshape(BSP, DN, NI).transpose(2, 1, 0))  # [16,64,1025]

    # m3d: [128, 16*1040]: per e, rows 0..63 joint (b 0..1023), 64..127 marg
    m3dd = np.zeros((128, NI, 1040), np.float32)
    m3dd[:DN, :, 0:BS] = m3t[:, :, 0:BS].transpose(1, 0, 2) * WSC
    m3dd[DN:, :, 0:BS] = m3t[:, :, 1:BS + 1].transpose(1, 0, 2) * WSC
    m3dd = np.clip(m3dd.reshape(128, NI * 1040), -240, 240).astype(F8)

    def dbl8(aT, pad):
        K, N = aT.shape
        out = np.zeros((4, 2, 128, pad), np.float32)
        out[:, :, :, :N] = aT.reshape(4, 2, 128, N) * WSC
        out = out.transpose(0, 2, 1, 3).reshape(4, 128, 2 * pad)
        return np.clip(out, -240.0, 240.0).astype(F8)

    return {
        "ytd": dbl8(ys.T, 1040),
        "mtd": dbl8(Ms.T, 1040),
        "m3d": m3dd,
    }


_LMASK = np.zeros(128, bool)
_GMASK = np.zeros(128, bool)
for _e in range(NI):
    for _p in range(2):
        _LMASK[32 * (_e % 4) + 4 * _p + _e // 4] = True
for _p in range(2):
    _GMASK[32 * _p + 8] = True


def combine_partials(accs):
    """accs: list of 8 [128, 8] fp32 arrays -> scalar loss (float64 math)."""
    a = np.stack([np.asarray(x, np.float64) for x in accs])  # [8,128,8]
    local = a[:, _LMASK, 0:2].sum()
    glob = a[:, _GMASK, 0:2].sum()
    return np.float32(BETA * local / (B * NI) + ALPHA * glob / B)


def make_in_maps(inputs):
    sh = _prep_shared(inputs)
    return [dict(sh, **_prep_core(inputs, c)) for c in range(NC)]


def get_runner():
    global _RUNNER
    if _RUNNER is None:
        _RUNNER = _build_nc()
    return _RUNNER


def kernel(**inputs) -> np.ndarray:
    from concourse.bass_utils import run_bass_kernel_spmd

    nc = get_runner()
    in_maps = make_in_maps(inputs)
    res = run_bass_kernel_spmd(nc, in_maps, list(range(NC)))
    return combine_partials([r["acc"] for r in res.results])


# revision 13
# speedup vs baseline: 1.2932x; 1.0090x over previous
"""DeepInfoMax loss kernel for 8 Trainium2 NeuronCores.

Strategy (hardcoded for B=8192, d=1024, n=16):
  - Data-parallel over batch: core c gets rows [c*1024, (c+1)*1024), plus ONE
    overlap row ((c+1)*1024 % B) of M so the global roll (M_prime) is exact.
  - Activations kept feature-major ([features, batch]) on-chip; fp8 DoubleRow
    matmuls with fp32 PSUM accumulation where FD>=256.
  - net(M) (phases A/B) and the experts' y-contribution are computed once and
    shared between the joint/marginal passes.
  - Expert pipeline: the y-part psum from the grouped first-layer matmul is
    kept in PSUM; the joint M3 contribution accumulates on top (K=64 fp8 MM),
    and a single [-A;+A] K=128 MM switches the psum to the marginal pass.
    Software-pipelined: expert e-1's second L2 pass fills the PE while expert
    e's h1 evictions run, hiding the psum WAR stalls.
  - All 36 scores (32 expert + 4 global) land on separate partition rows of
    two PSUM banks via PE column-tiling with masked +-w columns; one Softplus
    activation per bank (per-partition bias, accumulate) finishes the loss.
    Host sums the valid partition rows.
"""

import numpy as np
import ml_dtypes

B = 8192
D = 1024
NI = 16
DN = D // NI  # 64
NC = 8
BS = B // NC  # 1024
BSP = BS + 1  # 1025 (overlap col for the exact roll)
ALPHA = 0.5
BETA = 1.0

CH_P = [(0, 342), (342, 342), (684, 341)]
CH_C = [(0, 512), (512, 512)]

BF = ml_dtypes.bfloat16
F8 = ml_dtypes.float8_e4m3
WSC = 64.0

_RUNNER = None

# cstd column map (f32 consts)
C_GB0 = 0      # 8 cols: WSC*gb0 per m-tile
C_GB1 = 8      # 8 cols: WSC^2*gb1 per m-tile
C_LB1 = 16     # 16 cols: lb1 per expert
C_LB2 = 32     # 16 cols: lb2 per expert
C_L0B = 48
C_L1B = 49
C_SPB = 50     # softplus bias rows (local + global)
C_ZERO = 51
NCST = 52

# cbfd column map (bf16 consts)
B_W3M = 0      # 32 blocks x 8 cols: masked +-w3
B_W2G = 256    # 2 blocks x 9 cols: masked +-l2w (col 8 of each block)
B_L1W = 274    # 128 cols: l1w
NBF = 402


def _build_nc():
    import concourse.bass as bass  # noqa: F401
    import concourse.tile as tile
    import concourse.mybir as mybir
    from concourse import bacc
    from contextlib import ExitStack

    bf = mybir.dt.bfloat16
    f32 = mybir.dt.float32
    f8 = mybir.dt.float8e4
    AF = mybir.ActivationFunctionType
    OP = mybir.AluOpType
    DR = mybir.MatmulPerfMode.DoubleRow

    nc = bacc.Bacc()

    mtd = nc.dram_tensor("mtd", [4, 128, 2 * 1040], f8, kind="ExternalInput")
    ytd = nc.dram_tensor("ytd", [4, 128, 2 * 1040], f8, kind="ExternalInput")
    m3d = nc.dram_tensor("m3d", [128, 16 * 1040], f8, kind="ExternalInput")
    gw0d = nc.dram_tensor("gw0d", [4, 128, 2 * D], f8, kind="ExternalInput")
    gw1d = nc.dram_tensor("gw1d", [4, 128, 2 * D], f8, kind="ExternalInput")
    bxd = nc.dram_tensor("bxd", [4, 128, 2 * 2176], f8, kind="ExternalInput")
    acatd = nc.dram_tensor("acatd", [128, 16 * 256], f8, kind="ExternalInput")
    w2sp = nc.dram_tensor("w2sp", [128, 2048], bf, kind="ExternalInput")
    l0whd = nc.dram_tensor("l0whd", [128, 4 * 256], f8, kind="ExternalInput")
    cstd = nc.dram_tensor("cstd", [128, NCST], f32, kind="ExternalInput")
    cbfd = nc.dram_tensor("cbfd", [128, NBF], bf, kind="ExternalInput")
    acc = nc.dram_tensor("acc", [128, 8], f32, kind="ExternalOutput")

    IW = 1.0 / WSC
    IW2 = 1.0 / (WSC * WSC)

    with tile.TileContext(nc) as tc, ExitStack() as ctx:
        pconst = ctx.enter_context(tc.tile_pool(name="const", bufs=1))
        pgw = ctx.enter_context(tc.tile_pool(name="gw", bufs=8))
        pi8 = ctx.enter_context(tc.tile_pool(name="i8", bufs=16))
        pbx = ctx.enter_context(tc.tile_pool(name="bx", bufs=4))
        pac = ctx.enter_context(tc.tile_pool(name="ac", bufs=1))
        pze = ctx.enter_context(tc.tile_pool(name="ze", bufs=1))
        ph1 = ctx.enter_context(tc.tile_pool(name="h1", bufs=6))
        ph2 = ctx.enter_context(tc.tile_pool(name="h2", bufs=12))
        phg = ctx.enter_context(tc.tile_pool(name="hg", bufs=2))
        pex = ctx.enter_context(tc.tile_pool(name="ex", bufs=2))
        ppm = ctx.enter_context(tc.tile_pool(name="pm", bufs=6, space="PSUM"))
        pps = ctx.enter_context(tc.tile_pool(name="ps", bufs=1, space="PSUM"))

        # ---- phase A inputs on the two HWDGE rings ----
        mt_sb = [pi8.tile([128, 2 * 1040], f8, tag="i8", name=f"mt_{k}")
                 for k in range(4)]
        for k2 in range(4):
            nc.sync.dma_start(mt_sb[k2][:], mtd[k2, :, :])
        gw0_sb = [pgw.tile([128, 2 * D], f8, tag="gw", name=f"gw0_{k}")
                  for k in range(4)]
        for k2 in range(4):
            nc.scalar.dma_start(gw0_sb[k2][:], gw0d[k2, :, :])
        gw1_sb = [pgw.tile([128, 2 * D], f8, tag="gw", name=f"gw1_{k}")
                  for k in range(4)]
        for k2 in range(4):
            nc.scalar.dma_start(gw1_sb[k2][:], gw1d[k2, :, :])

        # ---- consts (gpsimd queue) ----
        cst = pconst.tile([128, NCST], f32, tag="cst")
        nc.gpsimd.dma_start(cst[:], cstd[:])
        cbf = pconst.tile([128, NBF], bf, tag="cbf")
        nc.gpsimd.dma_start(cbf[:], cbfd[:])
        l0wh_sb = pconst.tile([128, 4 * 256], f8, tag="l0wh")
        nc.gpsimd.dma_start(l0wh_sb[:], l0whd[:])
        acat_sb = pac.tile([128, 16 * 256], f8, tag="acat")
        nc.gpsimd.dma_start(acat_sb[:], acatd[:])
        w2s_sb = pac.tile([128, 2048], bf, tag="w2s")
        nc.gpsimd.dma_start(w2s_sb[:], w2sp[:])
        ze_sb = pze.tile([128, 16 * 1040], f8, tag="ze")
        nc.gpsimd.dma_start(ze_sb[:], m3d[:])
        acc_sb = pconst.tile([128, 8], f32, tag="acc")
        nc.vector.memset(acc_sb[:], 0.0)
        scr = pconst.tile([128, 1], f32, tag="scr")
        # dummy exp+ln: pull in the natural_log_exp table set before any real
        # ACT work so the tail softplus pays no table switch
        nc.scalar.activation(scr[:], cst[:, C_ZERO:C_ZERO + 1], AF.Exp)
        nc.scalar.activation(scr[:], cst[:, C_ZERO:C_ZERO + 1], AF.Ln,
                             bias=1.0)

        # ---- later-phase inputs on sync ring (no buf reuse -> no waits) ----
        yt_sb = [pi8.tile([128, 2 * 1040], f8, tag="i8", name=f"yt_{k}")
                 for k in range(4)]
        for k2 in range(4):
            nc.sync.dma_start(yt_sb[k2][:], ytd[k2, :, :])
        bx_sb = [pbx.tile([128, 2 * 2176], f8, tag="bx", name=f"bx_{k}")
                 for k in range(4)]
        for k2 in range(4):
            nc.sync.dma_start(bx_sb[k2][:], bxd[k2, :, :])

        # ---- phase A: hg = WSC*relu(M@gw0+gb0), fp8 DR pairs (ACT evict) ----
        hg_sb = [pi8.tile([128, 2 * 1040], f8, tag="i8", name=f"hg_{k}")
                 for k in range(4)]
        for m in range(8):
            for (c0, cw) in CH_P:
                ps = ppm.tile([128, 512], f32, tag="pm")
                for k2 in range(4):
                    nc.tensor.matmul(
                        ps[:, :cw],
                        gw0_sb[k2].rearrange("p (ko m) -> p ko m", ko=2)[
                            :, :, m * 128:(m + 1) * 128],
                        mt_sb[k2].rearrange("p (ko b) -> p ko b", ko=2)[
                            :, :, c0:c0 + cw],
                        start=(k2 == 0), stop=(k2 == 3), perf_mode=DR,
                    )
                nc.scalar.activation(
                    hg_sb[m // 2][:, (m % 2) * 1040 + c0:(m % 2) * 1040 + c0 + cw],
                    ps[:, :cw], AF.Relu,
                    bias=cst[:, C_GB0 + m:C_GB0 + m + 1], scale=IW,
                )

        # ---- phase B: hm = WSC*(hg@gw1+gb1), fp8 DR pairs (DVE evict) ----
        hm_sb = [pi8.tile([128, 2 * 1040], f8, tag="i8", name=f"hm_{k}")
                 for k in range(4)]
        for m in range(8):
            for (c0, cw) in CH_P:
                ps = ppm.tile([128, 512], f32, tag="pm")
                for k2 in range(4):
                    nc.tensor.matmul(
                        ps[:, :cw],
                        gw1_sb[k2].rearrange("p (ko m) -> p ko m", ko=2)[
                            :, :, m * 128:(m + 1) * 128],
                        hg_sb[k2].rearrange("p (ko b) -> p ko b", ko=2)[
                            :, :, c0:c0 + cw],
                        start=(k2 == 0), stop=(k2 == 3), perf_mode=DR,
                    )
                nc.vector.tensor_scalar(
                    hm_sb[m // 2][:, (m % 2) * 1040 + c0:(m % 2) * 1040 + c0 + cw],
                    ps[:, :cw], cst[:, C_GB1 + m:C_GB1 + m + 1], IW,
                    op0=OP.add, op1=OP.mult,
                )

        # score psum banks: rows 32j+4p+t = expert (e=4t+j, pass p);
        # rows 32p+8 = global pass p. One per batch-column chunk.
        ps_loc = [pps.tile([128, 512], f32, tag=f"S{ci}", name=f"S_{ci}")
                  for ci in range(2)]

        # ---- phase F: global discriminator (gy folded into psum),
        #      software-pipelined depth 2 ----
        FIT = [(p, ci) for p in range(2) for ci in range(2)]
        fh0 = [None] * 4
        fh1g = [None] * 4
        for it in range(6):
            if it < 4:
                p, ci = FIT[it]
                c0, cw = CH_C[ci]
                ps = ppm.tile([128, 512], f32, tag="pm")
                for k2 in range(4):
                    nc.tensor.matmul(
                        ps[:, :cw],
                        bx_sb[k2].rearrange("p (ko m) -> p ko m", ko=2)[
                            :, :, 16 * 128:17 * 128],
                        yt_sb[k2].rearrange("p (ko b) -> p ko b", ko=2)[
                            :, :, c0:c0 + cw],
                        start=(k2 == 0), stop=False, perf_mode=DR,
                    )
                for k2 in range(4):
                    nc.tensor.matmul(
                        ps[:, :cw],
                        l0wh_sb[:, k2 * 256:(k2 + 1) * 256].rearrange(
                            "p (ko m) -> p ko m", ko=2),
                        hm_sb[k2].rearrange("p (ko b) -> p ko b", ko=2)[
                            :, :, p + c0:p + c0 + cw],
                        start=False, stop=(k2 == 3), perf_mode=DR,
                    )
                h0 = phg.tile([128, 512], bf, tag="h0")
                nc.scalar.activation(
                    h0[:, :cw], ps[:, :cw], AF.Relu,
                    bias=cst[:, C_L0B:C_L0B + 1], scale=IW2)
                fh0[it] = h0
            if 1 <= it <= 4:
                pp_, ci_ = FIT[it - 1]
                c0, cw = CH_C[ci_]
                ps1 = ppm.tile([128, 512], f32, tag="pm")
                nc.tensor.matmul(
                    ps1[:, :cw], cbf[:, B_L1W:B_L1W + 128],
                    fh0[it - 1][:, :cw], start=True, stop=True)
                h1g = phg.tile([128, 512], bf, tag="h1g")
                nc.scalar.activation(
                    h1g[:, :cw], ps1[:, :cw], AF.Relu,
                    bias=cst[:, C_L1B:C_L1B + 1])
                fh1g[it - 1] = h1g
            if 2 <= it:
                pp_, ci_ = FIT[it - 2]
                c0, cw = CH_C[ci_]
                nc.tensor.matmul(
                    ps_loc[ci_][32 * pp_:32 * pp_ + 9, :cw],
                    cbf[:, B_W2G + pp_ * 9:B_W2G + (pp_ + 1) * 9],
                    fh1g[it - 2][:, :cw],
                    start=True, stop=True,
                    tile_position=(0, 32 * pp_), skip_group_check=True,
                )

        # ---- expert loop, software-pipelined ----
        h1_all = {}
        h2_tiles = {}

        def emit_L2(e, p):
            for ci, (c0, cw) in enumerate(CH_C):
                ps2 = ppm.tile([128, 512], f32, tag="pm")
                nc.tensor.matmul(
                    ps2[:, :cw],
                    w2s_sb[:, e * 128:(e + 1) * 128],
                    h1_all[(e, p)][:, c0:c0 + cw],
                    start=True, stop=True,
                )
                h2t = h2_tiles[(e, p)]
                nc.vector.tensor_scalar(
                    h2t[:, c0:c0 + cw], ps2[:, :cw],
                    cst[:, C_LB2 + e:C_LB2 + e + 1], 0.0,
                    op0=OP.add, op1=OP.max)

        def emit_burst(t):
            for ci, (c0, cw) in enumerate(CH_C):
                for p in range(2):
                    for j in range(4):
                        eb = 4 * t + j
                        blk = eb * 2 + p
                        nc.tensor.matmul(
                            ps_loc[ci][32 * j:32 * j + 8, :cw],
                            cbf[:, B_W3M + blk * 8:B_W3M + (blk + 1) * 8],
                            h2_tiles[(eb, p)][:, c0:c0 + cw],
                            # strips j>=2 see no F score MM; their first
                            # writer must reset the strip's has_written bits
                            start=(t == 0 and p == 0 and j >= 2),
                            stop=(t == 3 and p == 1),
                            tile_position=(0, 32 * j),
                            skip_group_check=True,
                        )

        for e in range(NI):
            for p in range(2):
                h1_all[(e, p)] = ph1.tile([128, BS], bf, tag="h1",
                                          name=f"h1_{e}_{p}")
                h2_tiles[(e, p)] = ph2.tile([128, BS], bf, tag="h2",
                                            name=f"h2_{e}_{p}")
            psC = []
            # stage 1: y-part + joint M3 into psum, evict h1 pass 0
            for ci, (c0, cw) in enumerate(CH_C):
                ps = ppm.tile([128, 512], f32, tag="pm")
                psC.append(ps)
                for k2 in range(4):
                    nc.tensor.matmul(
                        ps[:, :cw],
                        bx_sb[k2].rearrange("p (ko m) -> p ko m", ko=2)[
                            :, :, e * 128:(e + 1) * 128],
                        yt_sb[k2].rearrange("p (ko b) -> p ko b", ko=2)[
                            :, :, c0:c0 + cw],
                        start=(k2 == 0), stop=(k2 == 3), perf_mode=DR,
                    )
                nc.tensor.matmul(
                    ps[:, :cw],
                    acat_sb[0:64, e * 256:e * 256 + 128],
                    ze_sb[0:64, e * 1040 + c0:e * 1040 + c0 + cw],
                    start=False, stop=False, skip_group_check=True,
                )
                nc.scalar.activation(
                    h1_all[(e, 0)][:, c0:c0 + cw], ps[:, :cw], AF.Relu,
                    bias=cst[:, C_LB1 + e:C_LB1 + e + 1], scale=IW2)
            # deferred L2 pass 1 of previous expert fills the PE while the
            # h1 evictions above drain
            if e >= 1:
                emit_L2(e - 1, 1)
            # stage 2: switch psum to the marginal pass, evict h1 pass 1
            for ci, (c0, cw) in enumerate(CH_C):
                nc.tensor.matmul(
                    psC[ci][:, :cw],
                    acat_sb[:, e * 256 + 128:e * 256 + 256],
                    ze_sb[:, e * 1040 + c0:e * 1040 + c0 + cw],
                    start=False, stop=True, skip_group_check=True,
                )
                nc.scalar.activation(
                    h1_all[(e, 1)][:, c0:c0 + cw], psC[ci][:, :cw], AF.Relu,
                    bias=cst[:, C_LB1 + e:C_LB1 + e + 1], scale=IW2)
            emit_L2(e, 0)
            if e % 4 == 0 and e >= 4:
                emit_burst(e // 4 - 1)

        emit_L2(NI - 1, 1)
        # final burst + softplus reduction, pipelined by chunk
        t = 3
        for ci, (c0, cw) in enumerate(CH_C):
            for p in range(2):
                for j in range(4):
                    eb = 4 * t + j
                    blk = eb * 2 + p
                    nc.tensor.matmul(
                        ps_loc[ci][32 * j:32 * j + 8, :cw],
                        cbf[:, B_W3M + blk * 8:B_W3M + (blk + 1) * 8],
                        h2_tiles[(eb, p)][:, c0:c0 + cw],
                        start=False, stop=(p == 1),
                        tile_position=(0, 32 * j),
                        skip_group_check=True,
                    )
            ex_t = pex.tile([128, 512], f32, tag="ex", name=f"ex{ci}")
            nc.scalar.activation(
                ex_t[:], ps_loc[ci][:], AF.Exp,
                bias=cst[:, C_SPB:C_SPB + 1])
            spl = pex.tile([128, 512], f32, tag="ex", name=f"spl{ci}")
            nc.scalar.activation(
                spl[:], ex_t[:], AF.Ln, bias=1.0,
                accum_out=acc_sb[:, ci:ci + 1])

        nc.sync.dma_start(acc[:], acc_sb[:])

    nc.finalize()
    return nc


def _prep_shared(inputs):
    """Weight repack (identical for all cores)."""
    f32 = np.float32
    gw0 = np.asarray(inputs["gw0"], f32)
    gw1 = np.asarray(inputs["gw1"], f32)
    l0w = np.asarray(inputs["l0w"], f32)
    l1w = np.asarray(inputs["l1w"], f32)
    l2w = np.asarray(inputs["l2w"], f32)
    lW1 = np.asarray(inputs["lW1"], f32)
    lW2 = np.asarray(inputs["lW2"], f32)
    lW3 = np.asarray(inputs["lW3"], f32)
    gb0 = np.asarray(inputs["gb0"], f32)
    gb1 = np.asarray(inputs["gb1"], f32)
    l0b = np.asarray(inputs["l0b"], f32)
    l1b = np.asarray(inputs["l1b"], f32)
    l2b = np.asarray(inputs["l2b"], f32)
    lb1 = np.asarray(inputs["lb1"], f32)
    lb2 = np.asarray(inputs["lb2"], f32)
    lb3 = np.asarray(inputs["lb3"], f32)

    def dbl(a, scale=1.0, pad=None):
        K, N = a.shape
        Np = N if pad is None else pad
        out = np.zeros((4, 2, 128, Np), np.float32)
        out[:, :, :, :N] = a.reshape(4, 2, 128, N) * scale
        out = out.transpose(0, 2, 1, 3).reshape(4, 128, 2 * Np)
        return np.clip(out, -240.0, 240.0).astype(F8)

    acat = np.zeros((128, NI * 256), np.float32)
    for e in range(NI):
        A = lW1[e, :DN, :] * WSC
        acat[:DN, e * 256:e * 256 + 128] = A
        acat[:DN, e * 256 + 128:e * 256 + 256] = -A
        acat[DN:, e * 256 + 128:e * 256 + 256] = A
    acat = np.clip(acat, -240, 240).astype(F8)

    cbf = np.zeros((128, NBF), f32)
    for e in range(NI):
        for p in range(2):
            blk = e * 2 + p
            s = 4 * p + e // 4
            sgn = -1.0 if p == 0 else 1.0
            cbf[:, B_W3M + blk * 8 + s] = sgn * lW3[e, :, 0]
    for p in range(2):
        sgn = -1.0 if p == 0 else 1.0
        cbf[:, B_W2G + p * 9 + 8] = sgn * l2w[:, 0]
    cbf[:, B_L1W:B_L1W + 128] = l1w

    cst = np.zeros((128, NCST), f32)
    cst[:, C_GB0:C_GB0 + 8] = gb0.reshape(8, 128).T * WSC
    cst[:, C_GB1:C_GB1 + 8] = gb1.reshape(8, 128).T * (WSC * WSC)
    cst[:, C_LB1:C_LB1 + NI] = lb1.T
    cst[:, C_LB2:C_LB2 + NI] = lb2.T
    for e in range(NI):
        j, t = e % 4, e // 4
        for p in range(2):
            sgn = -1.0 if p == 0 else 1.0
            cst[32 * j + 4 * p + t, C_SPB] = sgn * lb3[e, 0]
    for p in range(2):
        sgn = -1.0 if p == 0 else 1.0
        cst[32 * p + 8, C_SPB] = sgn * l2b[0]
    cst[:, C_L0B] = l0b
    cst[:, C_L1B] = l1b

    l0wh = l0w[D:].reshape(4, 2, 128, 128) * WSC
    l0wh = np.clip(l0wh.transpose(2, 0, 1, 3).reshape(128, 4 * 256),
                   -240, 240).astype(F8)

    bcatx = np.concatenate(
        [lW1[:, DN:, :].transpose(1, 0, 2).reshape(D, NI * 128), l0w[:D]],
        axis=1)
    return {
        "gw0d": dbl(gw0, WSC),
        "gw1d": dbl(gw1, WSC),
        "bxd": dbl(bcatx, WSC, pad=2176),
        "acatd": acat,
        "w2sp": np.ascontiguousarray(
            lW2.transpose(1, 0, 2).reshape(128, NI * 128)).astype(BF),
        "l0whd": l0wh,
        "cstd": cst,
        "cbfd": cbf.astype(BF),
    }


def _prep_core(inputs, c):
    f32 = np.float32
    y = np.asarray(inputs["y"], f32)
    M = np.asarray(inputs["M"], f32)
    r0 = c * BS
    rows = np.arange(r0, r0 + BSP) % B
    Ms = M[rows]
    ys = y[r0:r0 + BS]
    m3t = np.ascontiguousarray(
        Ms.reshape(BSP, DN, NI).transpose(2, 1, 0))  # [16,64,1025]

    # m3d: [128, 16*1040]: per e, rows 0..63 joint (b 0..1023), 64..127 marg
    m3dd = np.zeros((128, NI, 1040), np.float32)
    m3dd[:DN, :, 0:BS] = m3t[:, :, 0:BS].transpose(1, 0, 2) * WSC
    m3dd[DN:, :, 0:BS] = m3t[:, :, 1:BS + 1].transpose(1, 0, 2) * WSC
    m3dd = np.clip(m3dd.reshape(128, NI * 1040), -240, 240).astype(F8)

    def dbl8(aT, pad):
        K, N = aT.shape
        out = np.zeros((4, 2, 128, pad), np.float32)
        out[:, :, :, :N] = aT.reshape(4, 2, 128, N) * WSC
        out = out.transpose(0, 2, 1, 3).reshape(4, 128, 2 * pad)
        return np.clip(out, -240.0, 240.0).astype(F8)

    return {
        "ytd": dbl8(ys.T, 1040),
        "mtd": dbl8(Ms.T, 1040),
        "m3d": m3dd,
    }


_LMASK = np.zeros(128, bool)
_GMASK = np.zeros(128, bool)
for _e in range(NI):
    for _p in range(2):
        _LMASK[32 * (_e % 4) + 4 * _p + _e // 4] = True
for _p in range(2):
    _GMASK[32 * _p + 8] = True


def combine_partials(accs):
    """accs: list of 8 [128, 8] fp32 arrays -> scalar loss (float64 math)."""
    a = np.stack([np.asarray(x, np.float64) for x in accs])  # [8,128,8]
    local = a[:, _LMASK, 0:2].sum()
    glob = a[:, _GMASK, 0:2].sum()
    return np.float32(BETA * local / (B * NI) + ALPHA * glob / B)


def make_in_maps(inputs):
    sh = _prep_shared(inputs)
    return [dict(sh, **_prep_core(inputs, c)) for c in range(NC)]


def get_runner():
    global _RUNNER
    if _RUNNER is None:
        _RUNNER = _build_nc()
    return _RUNNER


def kernel(**inputs) -> np.ndarray:
    from concourse.bass_utils import run_bass_kernel_spmd

    nc = get_runner()
    in_maps = make_in_maps(inputs)
    res = run_bass_kernel_spmd(nc, in_maps, list(range(NC)))
    return combine_partials([r["acc"] for r in res.results])


# revision 15
# speedup vs baseline: 1.2972x; 1.0031x over previous
"""DeepInfoMax loss kernel for 8 Trainium2 NeuronCores.

Strategy (hardcoded for B=8192, d=1024, n=16):
  - Data-parallel over batch: core c gets rows [c*1024, (c+1)*1024), plus ONE
    overlap row ((c+1)*1024 % B) of M so the global roll (M_prime) is exact.
  - Activations kept feature-major ([features, batch]) on-chip; fp8 DoubleRow
    matmuls with fp32 PSUM accumulation where FD>=256.
  - net(M) (phases A/B) and the experts' y-contribution are computed once and
    shared between the joint/marginal passes.
  - Expert pipeline: the y-part psum from the grouped first-layer matmul is
    kept in PSUM; the joint M3 contribution accumulates on top (K=64 fp8 MM),
    and a single [-A;+A] K=128 MM switches the psum to the marginal pass.
    Software-pipelined: expert e-1's second L2 pass fills the PE while expert
    e's h1 evictions run, hiding the psum WAR stalls.
  - All 36 scores (32 expert + 4 global) land on separate partition rows of
    two PSUM banks via PE column-tiling with masked +-w columns; one Softplus
    activation per bank (per-partition bias, accumulate) finishes the loss.
    Host sums the valid partition rows.
"""

import numpy as np
import ml_dtypes

B = 8192
D = 1024
NI = 16
DN = D // NI  # 64
NC = 8
BS = B // NC  # 1024
BSP = BS + 1  # 1025 (overlap col for the exact roll)
ALPHA = 0.5
BETA = 1.0

CH_P = [(0, 342), (342, 342), (684, 341)]
CH_C = [(0, 512), (512, 512)]

BF = ml_dtypes.bfloat16
F8 = ml_dtypes.float8_e4m3
WSC = 64.0

_RUNNER = None

# cstd column map (f32 consts)
C_GB0 = 0      # 8 cols: WSC*gb0 per m-tile
C_GB1 = 8      # 8 cols: WSC^2*gb1 per m-tile
C_LB1 = 16     # 16 cols: lb1 per expert
C_LB2 = 32     # 16 cols: lb2 per expert
C_L0B = 48
C_L1B = 49
C_SPB = 50     # softplus bias rows (local + global)
C_ZERO = 51
NCST = 52

# cbfd column map (bf16 consts)
B_W3M = 0      # 32 blocks x 8 cols: masked +-w3
B_W2G = 256    # 2 blocks x 9 cols: masked +-l2w (col 8 of each block)
B_L1W = 274    # 128 cols: l1w
B_ZW = 402     # 128 zero cols (score-bank init)
NBF = 530


def _build_nc():
    import concourse.bass as bass  # noqa: F401
    import concourse.tile as tile
    import concourse.mybir as mybir
    from concourse import bacc
    from contextlib import ExitStack

    bf = mybir.dt.bfloat16
    f32 = mybir.dt.float32
    f8 = mybir.dt.float8e4
    AF = mybir.ActivationFunctionType
    OP = mybir.AluOpType
    DR = mybir.MatmulPerfMode.DoubleRow

    nc = bacc.Bacc()

    mtd = nc.dram_tensor("mtd", [4, 128, 2 * 1040], f8, kind="ExternalInput")
    ytd = nc.dram_tensor("ytd", [4, 128, 2 * 1040], f8, kind="ExternalInput")
    m3d = nc.dram_tensor("m3d", [128, 16 * 1040], f8, kind="ExternalInput")
    gw0d = nc.dram_tensor("gw0d", [4, 128, 2 * D], f8, kind="ExternalInput")
    gw1d = nc.dram_tensor("gw1d", [4, 128, 2 * D], f8, kind="ExternalInput")
    bxd = nc.dram_tensor("bxd", [4, 128, 2 * 2176], f8, kind="ExternalInput")
    acatd = nc.dram_tensor("acatd", [128, 16 * 256], f8, kind="ExternalInput")
    w2sp = nc.dram_tensor("w2sp", [128, 2048], bf, kind="ExternalInput")
    l0whd = nc.dram_tensor("l0whd", [128, 4 * 256], f8, kind="ExternalInput")
    cstd = nc.dram_tensor("cstd", [128, NCST], f32, kind="ExternalInput")
    cbfd = nc.dram_tensor("cbfd", [128, NBF], bf, kind="ExternalInput")
    acc = nc.dram_tensor("acc", [128, 8], f32, kind="ExternalOutput")

    IW = 1.0 / WSC
    IW2 = 1.0 / (WSC * WSC)

    with tile.TileContext(nc) as tc, ExitStack() as ctx:
        pconst = ctx.enter_context(tc.tile_pool(name="const", bufs=1))
        pgw = ctx.enter_context(tc.tile_pool(name="gw", bufs=8))
        pi8 = ctx.enter_context(tc.tile_pool(name="i8", bufs=16))
        pbx = ctx.enter_context(tc.tile_pool(name="bx", bufs=4))
        pac = ctx.enter_context(tc.tile_pool(name="ac", bufs=1))
        pze = ctx.enter_context(tc.tile_pool(name="ze", bufs=1))
        ph1 = ctx.enter_context(tc.tile_pool(name="h1", bufs=6))
        ph2 = ctx.enter_context(tc.tile_pool(name="h2", bufs=12))
        phg = ctx.enter_context(tc.tile_pool(name="hg", bufs=2))
        pex = ctx.enter_context(tc.tile_pool(name="ex", bufs=2))
        ppm = ctx.enter_context(tc.tile_pool(name="pm", bufs=6, space="PSUM"))
        pps = ctx.enter_context(tc.tile_pool(name="ps", bufs=1, space="PSUM"))

        # ---- phase A inputs on the two HWDGE rings ----
        mt_sb = [pi8.tile([128, 2 * 1040], f8, tag="i8", name=f"mt_{k}")
                 for k in range(4)]
        for k2 in range(4):
            nc.sync.dma_start(mt_sb[k2][:], mtd[k2, :, :])
        gw0_sb = [pgw.tile([128, 2 * D], f8, tag="gw", name=f"gw0_{k}")
                  for k in range(4)]
        for k2 in range(4):
            nc.scalar.dma_start(gw0_sb[k2][:], gw0d[k2, :, :])
        gw1_sb = [pgw.tile([128, 2 * D], f8, tag="gw", name=f"gw1_{k}")
                  for k in range(4)]
        for k2 in range(4):
            nc.scalar.dma_start(gw1_sb[k2][:], gw1d[k2, :, :])

        # ---- consts (gpsimd queue) ----
        cst = pconst.tile([128, NCST], f32, tag="cst")
        nc.gpsimd.dma_start(cst[:], cstd[:])
        cbf = pconst.tile([128, NBF], bf, tag="cbf")
        nc.gpsimd.dma_start(cbf[:], cbfd[:])
        l0wh_sb = pconst.tile([128, 4 * 256], f8, tag="l0wh")
        nc.gpsimd.dma_start(l0wh_sb[:], l0whd[:])
        acat_sb = pac.tile([128, 16 * 256], f8, tag="acat")
        nc.gpsimd.dma_start(acat_sb[:], acatd[:])
        w2s_sb = pac.tile([128, 2048], bf, tag="w2s")
        nc.gpsimd.dma_start(w2s_sb[:], w2sp[:])
        ze_sb = pze.tile([128, 16 * 1040], f8, tag="ze")
        nc.gpsimd.dma_start(ze_sb[:], m3d[:])
        acc_sb = pconst.tile([128, 8], f32, tag="acc")
        nc.vector.memset(acc_sb[:], 0.0)
        scr = pconst.tile([128, 1], f32, tag="scr")
        # dummy exp+ln: pull in the natural_log_exp table set before any real
        # ACT work so the tail softplus pays no table switch
        nc.scalar.activation(scr[:], cst[:, C_ZERO:C_ZERO + 1], AF.Exp)
        nc.scalar.activation(scr[:], cst[:, C_ZERO:C_ZERO + 1], AF.Ln,
                             bias=1.0)

        # ---- later-phase inputs on sync ring (no buf reuse -> no waits) ----
        yt_sb = [pi8.tile([128, 2 * 1040], f8, tag="i8", name=f"yt_{k}")
                 for k in range(4)]
        for k2 in range(4):
            nc.sync.dma_start(yt_sb[k2][:], ytd[k2, :, :])
        bx_sb = [pbx.tile([128, 2 * 2176], f8, tag="bx", name=f"bx_{k}")
                 for k in range(4)]
        for k2 in range(4):
            nc.sync.dma_start(bx_sb[k2][:], bxd[k2, :, :])

        # ---- phase A: hg = WSC*relu(M@gw0+gb0), fp8 DR pairs (ACT evict) ----
        hg_sb = [pi8.tile([128, 2 * 1040], f8, tag="i8", name=f"hg_{k}")
                 for k in range(4)]
        for m in range(8):
            for (c0, cw) in CH_P:
                ps = ppm.tile([128, 512], f32, tag="pm")
                for k2 in range(4):
                    nc.tensor.matmul(
                        ps[:, :cw],
                        gw0_sb[k2].rearrange("p (ko m) -> p ko m", ko=2)[
                            :, :, m * 128:(m + 1) * 128],
                        mt_sb[k2].rearrange("p (ko b) -> p ko b", ko=2)[
                            :, :, c0:c0 + cw],
                        start=(k2 == 0), stop=(k2 == 3), perf_mode=DR,
                    )
                nc.scalar.activation(
                    hg_sb[m // 2][:, (m % 2) * 1040 + c0:(m % 2) * 1040 + c0 + cw],
                    ps[:, :cw], AF.Relu,
                    bias=cst[:, C_GB0 + m:C_GB0 + m + 1], scale=IW,
                )

        # ---- phase B: hm = WSC*(hg@gw1+gb1), fp8 DR pairs (DVE evict) ----
        hm_sb = [pi8.tile([128, 2 * 1040], f8, tag="i8", name=f"hm_{k}")
                 for k in range(4)]
        for m in range(8):
            for (c0, cw) in CH_P:
                ps = ppm.tile([128, 512], f32, tag="pm")
                for k2 in range(4):
                    nc.tensor.matmul(
                        ps[:, :cw],
                        gw1_sb[k2].rearrange("p (ko m) -> p ko m", ko=2)[
                            :, :, m * 128:(m + 1) * 128],
                        hg_sb[k2].rearrange("p (ko b) -> p ko b", ko=2)[
                            :, :, c0:c0 + cw],
                        start=(k2 == 0), stop=(k2 == 3), perf_mode=DR,
                    )
                nc.vector.tensor_scalar(
                    hm_sb[m // 2][:, (m % 2) * 1040 + c0:(m % 2) * 1040 + c0 + cw],
                    ps[:, :cw], cst[:, C_GB1 + m:C_GB1 + m + 1], IW,
                    op0=OP.add, op1=OP.mult,
                )

        # score psum banks: rows 32j+4p+t = expert (e=4t+j, pass p);
        # rows 32p+8 = global pass p. One per batch-column chunk.
        ps_loc = [pps.tile([128, 512], f32, tag=f"S{ci}", name=f"S_{ci}")
                  for ci in range(2)]
        for ci in range(2):
            nc.tensor.matmul(
                ps_loc[ci][:, :], cbf[:, B_ZW:B_ZW + 128],
                w2s_sb[:, 0:512], start=True, stop=False,
                skip_group_check=True,
            )

        # ---- phase F: global discriminator (gy folded into psum),
        #      software-pipelined depth 2 ----
        FIT = [(p, ci) for p in range(2) for ci in range(2)]
        fh0 = [None] * 4
        fh1g = [None] * 4
        for it in range(6):
            if it < 4:
                p, ci = FIT[it]
                c0, cw = CH_C[ci]
                ps = ppm.tile([128, 512], f32, tag="pm")
                for k2 in range(4):
                    nc.tensor.matmul(
                        ps[:, :cw],
                        bx_sb[k2].rearrange("p (ko m) -> p ko m", ko=2)[
                            :, :, 16 * 128:17 * 128],
                        yt_sb[k2].rearrange("p (ko b) -> p ko b", ko=2)[
                            :, :, c0:c0 + cw],
                        start=(k2 == 0), stop=False, perf_mode=DR,
                    )
                for k2 in range(4):
                    nc.tensor.matmul(
                        ps[:, :cw],
                        l0wh_sb[:, k2 * 256:(k2 + 1) * 256].rearrange(
                            "p (ko m) -> p ko m", ko=2),
                        hm_sb[k2].rearrange("p (ko b) -> p ko b", ko=2)[
                            :, :, p + c0:p + c0 + cw],
                        start=False, stop=(k2 == 3), perf_mode=DR,
                    )
                h0 = phg.tile([128, 512], bf, tag="h0")
                nc.scalar.activation(
                    h0[:, :cw], ps[:, :cw], AF.Relu,
                    bias=cst[:, C_L0B:C_L0B + 1], scale=IW2)
                fh0[it] = h0
            if 1 <= it <= 4:
                pp_, ci_ = FIT[it - 1]
                c0, cw = CH_C[ci_]
                ps1 = ppm.tile([128, 512], f32, tag="pm")
                nc.tensor.matmul(
                    ps1[:, :cw], cbf[:, B_L1W:B_L1W + 128],
                    fh0[it - 1][:, :cw], start=True, stop=True)
                h1g = phg.tile([128, 512], bf, tag="h1g")
                nc.scalar.activation(
                    h1g[:, :cw], ps1[:, :cw], AF.Relu,
                    bias=cst[:, C_L1B:C_L1B + 1])
                fh1g[it - 1] = h1g
            if 2 <= it:
                pp_, ci_ = FIT[it - 2]
                c0, cw = CH_C[ci_]
                nc.tensor.matmul(
                    ps_loc[ci_][32 * pp_:32 * pp_ + 9, :cw],
                    cbf[:, B_W2G + pp_ * 9:B_W2G + (pp_ + 1) * 9],
                    fh1g[it - 2][:, :cw],
                    start=False, stop=False,
                    tile_position=(0, 32 * pp_), skip_group_check=True,
                )

        # ---- expert loop, software-pipelined ----
        h1_all = {}
        h2_tiles = {}

        def emit_L2(e, p):
            for ci, (c0, cw) in enumerate(CH_C):
                ps2 = ppm.tile([128, 512], f32, tag="pm")
                nc.tensor.matmul(
                    ps2[:, :cw],
                    w2s_sb[:, e * 128:(e + 1) * 128],
                    h1_all[(e, p)][:, c0:c0 + cw],
                    start=True, stop=True,
                )
                h2t = h2_tiles[(e, p)]
                nc.vector.tensor_scalar(
                    h2t[:, c0:c0 + cw], ps2[:, :cw],
                    cst[:, C_LB2 + e:C_LB2 + e + 1], 0.0,
                    op0=OP.add, op1=OP.max)

        def emit_burst(t):
            for ci, (c0, cw) in enumerate(CH_C):
                for p in range(2):
                    for j in range(4):
                        eb = 4 * t + j
                        blk = eb * 2 + p
                        nc.tensor.matmul(
                            ps_loc[ci][32 * j:32 * j + 8, :cw],
                            cbf[:, B_W3M + blk * 8:B_W3M + (blk + 1) * 8],
                            h2_tiles[(eb, p)][:, c0:c0 + cw],
                            start=False,
                            stop=(t == 3 and p == 1),
                            tile_position=(0, 32 * j),
                            skip_group_check=True,
                        )

        for e in range(NI):
            for p in range(2):
                h1_all[(e, p)] = ph1.tile([128, BS], bf, tag="h1",
                                          name=f"h1_{e}_{p}")
                h2_tiles[(e, p)] = ph2.tile([128, BS], bf, tag="h2",
                                            name=f"h2_{e}_{p}")
            psC = []
            # stage 1: y-part + joint M3 into psum, evict h1 pass 0
            for ci, (c0, cw) in enumerate(CH_C):
                ps = ppm.tile([128, 512], f32, tag="pm")
                psC.append(ps)
                for k2 in range(4):
                    nc.tensor.matmul(
                        ps[:, :cw],
                        bx_sb[k2].rearrange("p (ko m) -> p ko m", ko=2)[
                            :, :, e * 128:(e + 1) * 128],
                        yt_sb[k2].rearrange("p (ko b) -> p ko b", ko=2)[
                            :, :, c0:c0 + cw],
                        start=(k2 == 0), stop=(k2 == 3), perf_mode=DR,
                    )
                nc.tensor.matmul(
                    ps[:, :cw],
                    acat_sb[0:64, e * 256:e * 256 + 128],
                    ze_sb[0:64, e * 1040 + c0:e * 1040 + c0 + cw],
                    start=False, stop=False, skip_group_check=True,
                )
                nc.scalar.activation(
                    h1_all[(e, 0)][:, c0:c0 + cw], ps[:, :cw], AF.Relu,
                    bias=cst[:, C_LB1 + e:C_LB1 + e + 1], scale=IW2)
            # deferred L2 pass 1 of previous expert fills the PE while the
            # h1 evictions above drain
            if e >= 1:
                emit_L2(e - 1, 1)
            # stage 2: switch psum to the marginal pass, evict h1 pass 1
            for ci, (c0, cw) in enumerate(CH_C):
                nc.tensor.matmul(
                    psC[ci][:, :cw],
                    acat_sb[:, e * 256 + 128:e * 256 + 256],
                    ze_sb[:, e * 1040 + c0:e * 1040 + c0 + cw],
                    start=False, stop=True, skip_group_check=True,
                )
                nc.scalar.activation(
                    h1_all[(e, 1)][:, c0:c0 + cw], psC[ci][:, :cw], AF.Relu,
                    bias=cst[:, C_LB1 + e:C_LB1 + e + 1], scale=IW2)
            emit_L2(e, 0)
            if e % 4 == 0 and e >= 4:
                emit_burst(e // 4 - 1)

        emit_L2(NI - 1, 1)
        # final burst + softplus reduction, pipelined by chunk
        t = 3
        for ci, (c0, cw) in enumerate(CH_C):
            for p in range(2):
                for j in range(4):
                    eb = 4 * t + j
                    blk = eb * 2 + p
                    nc.tensor.matmul(
                        ps_loc[ci][32 * j:32 * j + 8, :cw],
                        cbf[:, B_W3M + blk * 8:B_W3M + (blk + 1) * 8],
                        h2_tiles[(eb, p)][:, c0:c0 + cw],
                        start=False, stop=(p == 1),
                        tile_position=(0, 32 * j),
                        skip_group_check=True,
                    )
            ex_t = pex.tile([128, 512], f32, tag="ex", name=f"ex{ci}")
            nc.scalar.activation(
                ex_t[:], ps_loc[ci][:], AF.Exp,
                bias=cst[:, C_SPB:C_SPB + 1])
            spl = pex.tile([128, 512], f32, tag="ex", name=f"spl{ci}")
            nc.scalar.activation(
                spl[:], ex_t[:], AF.Ln, bias=1.0,
                accum_out=acc_sb[:, ci:ci + 1])

        nc.sync.dma_start(acc[:], acc_sb[:])

    nc.finalize()
    return nc


def _prep_shared(inputs):
    """Weight repack (identical for all cores)."""
    f32 = np.float32
    gw0 = np.asarray(inputs["gw0"], f32)
    gw1 = np.asarray(inputs["gw1"], f32)
    l0w = np.asarray(inputs["l0w"], f32)
    l1w = np.asarray(inputs["l1w"], f32)
    l2w = np.asarray(inputs["l2w"], f32)
    lW1 = np.asarray(inputs["lW1"], f32)
    lW2 = np.asarray(inputs["lW2"], f32)
    lW3 = np.asarray(inputs["lW3"], f32)
    gb0 = np.asarray(inputs["gb0"], f32)
    gb1 = np.asarray(inputs["gb1"], f32)
    l0b = np.asarray(inputs["l0b"], f32)
    l1b = np.asarray(inputs["l1b"], f32)
    l2b = np.asarray(inputs["l2b"], f32)
    lb1 = np.asarray(inputs["lb1"], f32)
    lb2 = np.asarray(inputs["lb2"], f32)
    lb3 = np.asarray(inputs["lb3"], f32)

    def dbl(a, scale=1.0, pad=None):
        K, N = a.shape
        Np = N if pad is None else pad
        out = np.zeros((4, 2, 128, Np), np.float32)
        out[:, :, :, :N] = a.reshape(4, 2, 128, N) * scale
        out = out.transpose(0, 2, 1, 3).reshape(4, 128, 2 * Np)
        return np.clip(out, -240.0, 240.0).astype(F8)

    acat = np.zeros((128, NI * 256), np.float32)
    for e in range(NI):
        A = lW1[e, :DN, :] * WSC
        acat[:DN, e * 256:e * 256 + 128] = A
        acat[:DN, e * 256 + 128:e * 256 + 256] = -A
        acat[DN:, e * 256 + 128:e * 256 + 256] = A
    acat = np.clip(acat, -240, 240).astype(F8)

    cbf = np.zeros((128, NBF), f32)
    for e in range(NI):
        for p in range(2):
            blk = e * 2 + p
            s = 4 * p + e // 4
            sgn = -1.0 if p == 0 else 1.0
            cbf[:, B_W3M + blk * 8 + s] = sgn * lW3[e, :, 0]
    for p in range(2):
        sgn = -1.0 if p == 0 else 1.0
        cbf[:, B_W2G + p * 9 + 8] = sgn * l2w[:, 0]
    cbf[:, B_L1W:B_L1W + 128] = l1w

    cst = np.zeros((128, NCST), f32)
    cst[:, C_GB0:C_GB0 + 8] = gb0.reshape(8, 128).T * WSC
    cst[:, C_GB1:C_GB1 + 8] = gb1.reshape(8, 128).T * (WSC * WSC)
    cst[:, C_LB1:C_LB1 + NI] = lb1.T
    cst[:, C_LB2:C_LB2 + NI] = lb2.T
    for e in range(NI):
        j, t = e % 4, e // 4
        for p in range(2):
            sgn = -1.0 if p == 0 else 1.0
            cst[32 * j + 4 * p + t, C_SPB] = sgn * lb3[e, 0]
    for p in range(2):
        sgn = -1.0 if p == 0 else 1.0
        cst[32 * p + 8, C_SPB] = sgn * l2b[0]
    cst[:, C_L0B] = l0b
    cst[:, C_L1B] = l1b

    l0wh = l0w[D:].reshape(4, 2, 128, 128) * WSC
    l0wh = np.clip(l0wh.transpose(2, 0, 1, 3).reshape(128, 4 * 256),
                   -240, 240).astype(F8)

    bcatx = np.concatenate(
        [lW1[:, DN:, :].transpose(1, 0, 2).reshape(D, NI * 128), l0w[:D]],
        axis=1)
    return {
        "gw0d": dbl(gw0, WSC),
        "gw1d": dbl(gw1, WSC),
        "bxd": dbl(bcatx, WSC, pad=2176),
        "acatd": acat,
        "w2sp": np.ascontiguousarray(
            lW2.transpose(1, 0, 2).reshape(128, NI * 128)).astype(BF),
        "l0whd": l0wh,
        "cstd": cst,
        "cbfd": cbf.astype(BF),
    }


def _prep_core(inputs, c):
    f32 = np.float32
    y = np.asarray(inputs["y"], f32)
    M = np.asarray(inputs["M"], f32)
    r0 = c * BS
    rows = np.arange(r0, r0 + BSP) % B
    Ms = M[rows]
    ys = y[r0:r0 + BS]
    m3t = np.ascontiguousarray(
        Ms.reshape(BSP, DN, NI).transpose(2, 1, 0))  # [16,64,1025]

    # m3d: [128, 16*1040]: per e, rows 0..63 joint (b 0..1023), 64..127 marg
    m3dd = np.zeros((128, NI, 1040), np.float32)
    m3dd[:DN, :, 0:BS] = m3t[:, :, 0:BS].transpose(1, 0, 2) * WSC
    m3dd[DN:, :, 0:BS] = m3t[:, :, 1:BS + 1].transpose(1, 0, 2) * WSC
    m3dd = np.clip(m3dd.reshape(128, NI * 1040), -240, 240).astype(F8)

    def dbl8(aT, pad):
        K, N = aT.shape
        out = np.zeros((4, 2, 128, pad), np.float32)
        out[:, :, :, :N] = aT.reshape(4, 2, 128, N) * WSC
        out = out.transpose(0, 2, 1, 3).reshape(4, 128, 2 * pad)
        return np.clip(out, -240.0, 240.0).astype(F8)

    return {
        "ytd": dbl8(ys.T, 1040),
        "mtd": dbl8(Ms.T, 1040),
        "m3d": m3dd,
    }


_LMASK = np.zeros(128, bool)
_GMASK = np.zeros(128, bool)
for _e in range(NI):
    for _p in range(2):
        _LMASK[32 * (_e % 4) + 4 * _p + _e // 4] = True
for _p in range(2):
    _GMASK[32 * _p + 8] = True


def combine_partials(accs):
    """accs: list of 8 [128, 8] fp32 arrays -> scalar loss (float64 math)."""
    a = np.stack([np.asarray(x, np.float64) for x in accs])  # [8,128,8]
    local = a[:, _LMASK, 0:2].sum()
    glob = a[:, _GMASK, 0:2].sum()
    return np.float32(BETA * local / (B * NI) + ALPHA * glob / B)


def make_in_maps(inputs):
    sh = _prep_shared(inputs)
    return [dict(sh, **_prep_core(inputs, c)) for c in range(NC)]


def get_runner():
    global _RUNNER
    if _RUNNER is None:
        _RUNNER = _build_nc()
    return _RUNNER


def kernel(**inputs) -> np.ndarray:
    from concourse.bass_utils import run_bass_kernel_spmd

    nc = get_runner()
    in_maps = make_in_maps(inputs)
    res = run_bass_kernel_spmd(nc, in_maps, list(range(NC)))
    return combine_partials([r["acc"] for r in res.results])
